# revision 16
# baseline (speedup 1.0000x reference)
"""Dynamic Directional Attention on 8 trn2 NeuronCores (Bass/Tile), v2.

Problem: B=4, L=S=2048, H=8, E=64, f32.
  qt = tanh(q * 1/(std_H(q)+eps) * dw) * dyn     (std over the HEAD dim, ddof=1)
  kt likewise; scores[b,h,l,s] = qt . kt          (contract E)
  tau[l] = sqrt(var_s(scores[l,:], ddof=1) + eps)
  A = softmax(scale * scores / tau);  out = A @ v  [B,L,H,E]

Sharding: 8 cores = 4 batches x 2 L-halves. Each core: q[b, half] = [1024, 512],
full k/v[b] = [2048, 512]. No collectives.

v2 changes vs baseline (375us):
  - q/k/v pre-cast to bf16 on host: DMA traffic halved, no input casts on chip.
  - A@V restructured V-stationary: lhsT = Vaug [s,65], rhs = A^T streaming
    N=512. Kills the 1024x ~110ns 128-col LDWEIGHTS the A-stationary layout
    paid (~110us of pure weight-load on the PE).
  - Output produced as outT [65, l] per head (row 64 = softmax denominator),
    denominator row reciprocated in place, PE-transposed back to [l, 65],
    scaled per-partition on DVE.
  - Transform stats batched: one Rsqrt activation over all 24 chunks' vars;
    table sets: rsqrt -> exp_and_others(tanh) -> rsqrt(m) -> exp (4 loads,
    vs 8 in the baseline).
  - ksum matmuls merged into the Gram matmuls via an ones column in the
    transformed-k tiles (pair-block layout [128, 2, 4, 130]).
  - PSUM->SBUF row-stat drains via DMA instead of DVE copies.
"""

import os
import sys

for _p in ("/opt/trn_rl_repo", "/root/.axon_site/_ro/trn_rl_repo"):
    if os.path.isdir(_p) and _p not in sys.path:
        sys.path.append(_p)

import numpy as np
import ml_dtypes

import concourse.bass as bass
import concourse.mybir as mybir
import concourse.tile as tile
from concourse import bacc
from concourse.bass_utils import run_bass_kernel_spmd
from concourse.masks import make_identity

F32 = mybir.dt.float32
BF16 = mybir.dt.bfloat16
AF = mybir.ActivationFunctionType
ALU = mybir.AluOpType

B, L, S, H, E = 4, 2048, 2048, 8, 64
LC = L // 2          # 1024 l-rows per core
D = H * E            # 512 free-dim columns per core
P = 128
NLT = LC // P        # 8 l-chunks
NST = S // P         # 16 s-chunks
LB = 512
NHP = H // 2         # 4 head-pairs
EV = E + 1           # 65: V block incl ones column
BLK = 130            # tn pair-block stride: 2*64 e-cols + ones + pad
NKG = 4              # at kk-group size
KGRP = NST // NKG    # 4 groups per head
EPS = 1e-6
SCALE = 1.0 / np.sqrt(E)
UNB_H = float(H) / float(H - 1)
UNB_S = float(S) / float(S - 1)

_last_exec_time_ns = None


def _ensure_axon_hooks():
    """Provide antenv.axon_hooks (NTFF profiling hook) if the image lacks it."""
    try:
        import antenv.axon_hooks  # noqa: F401

        return
    except ImportError:
        pass
    import contextlib
    import ctypes
    import types

    try:
        import antenv
    except ImportError:
        return

    holder = {"h": None}
    mod = types.ModuleType("antenv.axon_hooks")
    mod.set_axon_ntff_profile_hook = lambda h: holder.__setitem__("h", h)
    mod.get_axon_ntff_profile_hook = lambda: holder["h"]
    sys.modules["antenv.axon_hooks"] = mod
    antenv.axon_hooks = mod

    so_path = "/opt/axon/libaxon_pjrt.so"
    if not os.path.exists(so_path):
        return
    try:
        lib = ctypes.CDLL(so_path)
    except OSError:
        return
    if not hasattr(lib, "axon_start_nrt_profile"):
        return
    lib.axon_start_nrt_profile.argtypes = [
        ctypes.POINTER(ctypes.c_int64),
        ctypes.c_size_t,
    ]
    lib.axon_start_nrt_profile.restype = ctypes.c_int64
    lib.axon_stop_nrt_profile.argtypes = [ctypes.c_char_p]
    lib.axon_stop_nrt_profile.restype = ctypes.c_int64

    @contextlib.contextmanager
    def _hook(output_dir, device_ids):
        import jax

        jax.devices()
        if device_ids:
            ids = (ctypes.c_int64 * len(device_ids))(*device_ids)
            rc = lib.axon_start_nrt_profile(ids, len(device_ids))
        else:
            rc = lib.axon_start_nrt_profile(None, 0)
        if rc != 0:
            raise RuntimeError(f"axon_start_nrt_profile rc={rc}")
        try:
            yield
        finally:
            n = lib.axon_stop_nrt_profile(str(output_dir).encode())
            print(f"profile: {n} file(s) written to {output_dir}", file=sys.stderr)

    holder["h"] = _hook


def _head_bcast(ap_2d, nh=H, ne=E):
    """View a [p, ne] AP as [p, nh, ne] with the head dim broadcast (step 0)."""
    return bass.AP(
        tensor=ap_2d.tensor,
        offset=ap_2d.offset,
        ap=[list(ap_2d.ap[0]), [0, nh], list(ap_2d.ap[1])],
    )


def build_nc():
    nc = bacc.Bacc("TRN2", target_bir_lowering=False, debug=False)
    q_d = nc.dram_tensor("q", [LC, D], BF16, kind="ExternalInput")
    k_d = nc.dram_tensor("k", [S, D], BF16, kind="ExternalInput")
    v_d = nc.dram_tensor("v", [S, D], BF16, kind="ExternalInput")
    dw_d = nc.dram_tensor("dw", [1, 1], F32, kind="ExternalInput")
    dp_d = nc.dram_tensor("dp", [1, 1], F32, kind="ExternalInput")
    o_d = nc.dram_tensor("o", [LC, D], F32, kind="ExternalOutput")

    q_r = q_d.rearrange("(n p) d -> p n d", p=P)
    k_r = k_d.rearrange("(n p) d -> p n d", p=P)
    v_r = v_d.rearrange("(n p) d -> p n d", p=P)
    o_r = o_d.rearrange("(n p) d -> p n d", p=P)

    from contextlib import ExitStack

    with tile.TileContext(nc) as tc, ExitStack() as ctx:
        ek = ctx.enter_context
        sing = ek(tc.tile_pool(name="sing", bufs=1))
        pnat = ek(tc.tile_pool(name="nat", bufs=12))
        psq = ek(tc.tile_pool(name="sq", bufs=2))
        pstat = ek(tc.tile_pool(name="stat", bufs=1))
        ptn = ek(tc.tile_pool(name="tn", bufs=12))
        pqt = ek(tc.tile_pool(name="qt", bufs=4))
        pgw = ek(tc.tile_pool(name="gw", bufs=2))
        prw = ek(tc.tile_pool(name="rows", bufs=1))
        pdr = ek(tc.tile_pool(name="dr", bufs=1, space="DRAM"))
        pmb = ek(tc.tile_pool(name="mb", bufs=2))
        pqts = ek(tc.tile_pool(name="qts", bufs=4))
        pat = ek(tc.tile_pool(name="at", bufs=5))          # kk-group A^T bf16
        pva = ek(tc.tile_pool(name="va", bufs=1))
        pvn = ek(tc.tile_pool(name="vn", bufs=2))
        pobt = ek(tc.tile_pool(name="obt", bufs=4))        # [65,512] f32 outT
        posb = ek(tc.tile_pool(name="osb", bufs=2))
        pps = ek(tc.tile_pool(name="ps", bufs=2, space="PSUM"))    # [128,1024]
        ppo = ek(tc.tile_pool(name="po", bufs=2, space="PSUM"))    # [65,512]
        ppx = ek(tc.tile_pool(name="px", bufs=2, space="PSUM"))    # shared 2KB

        # --- constants ---
        ident = sing.tile([P, P], BF16)
        make_identity(nc, ident)
        identf = sing.tile([P, P], F32)
        make_identity(nc, identf)
        zero_t = sing.tile([P, 1], F32)
        nc.vector.memset(zero_t, 0.0)
        eps_t = sing.tile([P, 1], F32)
        nc.vector.memset(eps_t, EPS)
        dw_t = sing.tile([P, 1], F32)
        nc.sync.dma_start(out=dw_t, in_=dw_d[:, :].to_broadcast([P, 1]))
        dp_t = sing.tile([P, 1], F32)
        nc.sync.dma_start(out=dp_t, in_=dp_d[:, :].to_broadcast([P, 1]))
        dp2 = sing.tile([P, 1], F32)
        nc.vector.tensor_mul(dp2, dp_t, dp_t)
        c2 = sing.tile([P, 1], F32)  # scale * dyn^2
        nc.vector.tensor_scalar_mul(c2, dp2, float(SCALE))
        dp4 = sing.tile([P, 1], F32)
        nc.vector.tensor_mul(dp4, dp2, dp2)
        a_t = sing.tile([P, 1], F32)  # dyn^4 * UNB_S / S
        nc.vector.tensor_scalar_mul(a_t, dp4, UNB_S / S)
        b_t = sing.tile([P, 1], F32)  # dyn^4 * UNB_S / S^2
        nc.vector.tensor_scalar_mul(b_t, dp4, UNB_S / S / S)
        o2 = sing.tile([P, 2], BF16)  # per-head ones blocks for row sums
        nc.vector.memset(o2, 0.0)
        nc.vector.memset(o2[0:E, 0:1], 1.0)
        nc.vector.memset(o2[E:P, 1:2], 1.0)

        # --- V path: load v (bf16), pack Vaug [128, kk, h, 65] ---
        va = pva.tile([P, NST, H, EV], BF16)
        nc.vector.memset(
            va.rearrange("p a h c -> p (a h) c")[:, :, E : E + 1], 1.0)
        for kk in range(NST):
            vn = pvn.tile([P, D], BF16, tag="vn")
            nc.sync.dma_start(out=vn, in_=v_r[:, kk, :])
            nc.vector.tensor_copy(
                va[:, kk, :, 0:E],
                vn.rearrange("p (h e) -> p h e", h=H))

        # --- T1: load q/k nat pairs; per-chunk head sums into batched tiles ---
        NPAIR = (NLT + NST) // 2  # 12
        nat_pairs = []
        ssum_all = pstat.tile([P, NPAIR, 2, E], F32, tag="ssum")
        ssq_all = pstat.tile([P, NPAIR, 2, E], F32, tag="ssq")
        rstd_all = ssum_all  # reused once mean^2 is consumed by the var stt

        def t1_pair(src_r, i0, pi):
            nat = pnat.tile([P, 2, D], BF16, tag="nat")
            for u in range(2):
                nc.sync.dma_start(out=nat[:, u, :], in_=src_r[:, i0 + u, :])
            for u in range(2):
                sq = psq.tile([P, D], F32, tag="sq")
                nc.gpsimd.tensor_mul(sq, nat[:, u, :], nat[:, u, :])
                nc.vector.tensor_reduce(
                    ssum_all[:, pi, u, :],
                    nat[:, u, :].rearrange("p (h e) -> p e h", h=H),
                    axis=mybir.AxisListType.X, op=ALU.add)
                nc.vector.tensor_reduce(
                    ssq_all[:, pi, u, :],
                    sq.rearrange("p (h e) -> p e h", h=H),
                    axis=mybir.AxisListType.X, op=ALU.add)
            nat_pairs.append(nat)

        for j in range(NLT // 2):
            t1_pair(q_r, 2 * j, j)
        for j in range(NST // 2):
            t1_pair(k_r, 2 * j, NLT // 2 + j)

        # batched: mean^2 and biased var; rstd = rsqrt(UNB_H*var + eps)
        flat_sum = ssum_all.rearrange("p a b e -> p (a b e)")
        flat_sq = ssq_all.rearrange("p a b e -> p (a b e)")
        nc.vector.tensor_scalar_mul(flat_sum, flat_sum, 1.0 / H)
        nc.vector.tensor_mul(flat_sum, flat_sum, flat_sum)
        nc.vector.scalar_tensor_tensor(
            flat_sq, flat_sq, 1.0 / H, flat_sum,
            op0=ALU.mult, op1=ALU.subtract)
        # rstd = sqrt(1/(UNB_H*var + eps)) via fast DVE reciprocal + ACT Sqrt
        nc.vector.tensor_scalar(
            flat_sq, flat_sq, UNB_H, EPS, op0=ALU.mult, op1=ALU.add)
        flat_rstd = rstd_all.rearrange("p a b e -> p (a b e)")
        nc.vector.reciprocal_approx_fast(flat_rstd, flat_sq)
        nc.scalar.activation(flat_rstd, flat_rstd, AF.Sqrt,
                             bias=zero_t, scale=1.0)

        # --- T3: tmp = nat*rstd(bcast heads); tanh -> tn [128,2,4,130] ---
        tn_pairs = []
        for pi, nat in enumerate(nat_pairs):
            tn = ptn.tile([P, 2, NHP, BLK], BF16, tag="tn")
            for u in range(2):
                tmp = psq.tile([P, H, E], BF16, tag="tmp")
                nc.vector.tensor_mul(
                    tmp, nat[:, u, :].rearrange("p (h e) -> p h e", h=H),
                    _head_bcast(rstd_all[:, pi, u, :]))
                nc.scalar.activation(
                    tn[:, u, :, 0 : 2 * E], tmp.rearrange("p h e -> p (h e)"),
                    AF.Tanh, bias=zero_t, scale=dw_t)
            tn_pairs.append(tn)
        tn_q = tn_pairs[: NLT // 2]
        tn_k = tn_pairs[NLT // 2 :]
        for tn in tn_k:  # ones column feeding ksum via the Gram matmul
            nc.vector.memset(
                tn.rearrange("p u h c -> p (u h) c")[:, :, 2 * E : 2 * E + 1],
                1.0)

        # --- TR: PE-transpose tq/tk into [2E, l]/[2E, s] per pair ---
        tqT = [pqt.tile([P, LC], BF16, tag="tqT", name=f"tqT{i}")
               for i in range(NHP)]
        tkT = [pqt.tile([P, S], BF16, tag="tkT", name=f"tkT{i}")
               for i in range(NHP)]
        for tiles, dsts, npair in ((tn_q, tqT, NLT // 2), (tn_k, tkT, NST // 2)):
            for j in range(0, npair, 2):
                for hp in range(NHP):
                    pt = ppx.tile([P, 4, P], BF16, tag="px")
                    for w in range(2):
                        for u in range(2):
                            nc.tensor.transpose(
                                pt[:, 2 * w + u, :],
                                tiles[j + w][:, u, hp, 0 : 2 * E], ident)
                    nc.vector.tensor_copy(
                        dsts[hp][:, (2 * j) * P : (2 * j + 4) * P],
                        pt.rearrange("p a b -> p (a b)"))

        # --- G phase per pair: block-diag Gram + ksum column; row stats ---
        ssq_sb = prw.tile([8, LC], F32, tag="ssqsb")
        rsum_sb = prw.tile([8, LC], F32, tag="rsumsb")
        for hp in range(NHP):
            g_ps = ppx.tile([P, 2 * E + 1], F32, tag="px")
            for idx in range(NST):
                tn = tn_k[idx // 2]
                u = idx % 2
                nc.tensor.matmul(
                    g_ps, tn[:, u, hp, 0 : 2 * E], tn[:, u, hp, 0 : 2 * E + 1],
                    start=(idx == 0), stop=(idx == NST - 1))
            gsb = pgw.tile([P, P], BF16, tag="gsb")
            nc.vector.tensor_copy(gsb, g_ps[:, 0 : 2 * E])
            nc.vector.memset(gsb[0:E, E:P], 0.0)
            nc.vector.memset(gsb[E:P, 0:E], 0.0)
            k2 = pgw.tile([P, 2], BF16, tag="k2")
            nc.vector.memset(k2, 0.0)
            nc.vector.tensor_copy(k2[0:E, 0:1], g_ps[0:E, 2 * E : 2 * E + 1])
            nc.vector.tensor_copy(k2[E:P, 1:2], g_ps[E:P, 2 * E : 2 * E + 1])

            wps = pps.tile([P, LC], F32, tag="ps")
            for j in range(2):
                nc.tensor.matmul(wps[:, j * LB : (j + 1) * LB], gsb,
                                 tqT[hp][:, j * LB : (j + 1) * LB],
                                 start=True, stop=True)
            wsb = pgw.tile([P, LC], BF16, tag="wsb")
            nc.vector.tensor_copy(wsb, wps)
            prod = pgw.tile([P, LC], BF16, tag="prod")
            nc.vector.tensor_mul(prod, tqT[hp], wsb)
            stg_ss = pgw.tile([2, LC], F32, tag="stgss", bufs=1)
            stg_rs = pgw.tile([2, LC], F32, tag="stgrs", bufs=1)
            for j in range(2):
                r_ss = ppx.tile([2, LB], F32, tag="px")
                nc.tensor.matmul(r_ss, o2, prod[:, j * LB : (j + 1) * LB],
                                 start=True, stop=True)
                nc.vector.tensor_copy(stg_ss[:, j * LB : (j + 1) * LB], r_ss)
                r_rs = ppx.tile([2, LB], F32, tag="px")
                nc.tensor.matmul(r_rs, k2, tqT[hp][:, j * LB : (j + 1) * LB],
                                 start=True, stop=True)
                nc.vector.tensor_copy(stg_rs[:, j * LB : (j + 1) * LB], r_rs)
            nc.sync.dma_start(out=ssq_sb[2 * hp : 2 * hp + 2, :], in_=stg_ss)
            nc.sync.dma_start(out=rsum_sb[2 * hp : 2 * hp + 2, :], in_=stg_rs)

        # m = c2 * rsqrt(ssq*a - rsum^2*b + eps), vectorized over 8 heads
        nc.vector.tensor_mul(rsum_sb, rsum_sb, rsum_sb)
        nc.vector.tensor_scalar_mul(rsum_sb, rsum_sb, b_t[0:8, :])
        nc.vector.scalar_tensor_tensor(
            ssq_sb, ssq_sb, a_t[0:8, :], rsum_sb,
            op0=ALU.mult, op1=ALU.subtract)
        # m = c2 / sqrt(tau2 + eps): fast reciprocal then Sqrt
        nc.vector.tensor_scalar_add(ssq_sb, ssq_sb, EPS)
        nc.vector.reciprocal_approx_fast(rsum_sb, ssq_sb)
        nc.scalar.activation(rsum_sb, rsum_sb, AF.Sqrt,
                             bias=zero_t[0:8, :], scale=1.0)
        nc.vector.tensor_scalar_mul(rsum_sb, rsum_sb, c2[0:8, :])
        m8b = prw.tile([8, LC], BF16, tag="m8b")
        nc.vector.tensor_copy(m8b, rsum_sb)
        mdr = pdr.tile([8, LC], BF16, tag="mdr")
        nc.sync.dma_start(out=mdr[:, :], in_=m8b)

        # qts per pair: tq * m (broadcast m rows from DRAM)
        qts_l = []
        for hp in range(NHP):
            mb = pmb.tile([P, LC], BF16, tag="mb")
            for local in range(2):
                h = 2 * hp + local
                nc.sync.dma_start(out=mb[local * E : (local + 1) * E, :],
                                  in_=mdr[h : h + 1, :].to_broadcast([E, LC]))
            qts = pqts.tile([P, LC], BF16, tag="qts")
            nc.vector.tensor_mul(qts, tqT[hp], mb)
            qts_l.append(qts)

        # --- head loop: st -> exp -> V-stationary A@V -> untranspose,
        #     interleaved at kk-group granularity so PE alternates between
        #     st(h) and AV(h-1) while ACT streams exp(h) ---
        def emit_st_group(h, g):
            """st + exp for head h, kk group g; returns the at-group tile."""
            hp, local = h // 2, h % 2
            off = local * E
            tk = tkT[hp]
            at = pat.tile([P, NKG, LC], BF16, tag="at")
            for kg in range(NKG):
                kk = g * NKG + kg
                st_ps = pps.tile([P, LC], F32, tag="ps")
                for lb in range(2):
                    nc.tensor.matmul(
                        st_ps[:, lb * LB : (lb + 1) * LB],
                        tk[off : off + E, kk * P : (kk + 1) * P],
                        qts_l[hp][off : off + E, lb * LB : (lb + 1) * LB],
                        start=True, stop=True)
                nc.scalar.activation(at[:, kg, :], st_ps, AF.Exp,
                                     bias=zero_t, scale=1.0)
            return at

        def emit_av_group(h, g, at, pos):
            for kg in range(NKG):
                kk = g * NKG + kg
                for lb in range(2):
                    nc.tensor.matmul(
                        pos[lb], va[:, kk, h, :],
                        at[:, kg, lb * LB : (lb + 1) * LB],
                        start=(kk == 0), stop=(kk == NST - 1))

        def emit_av_end(pos):
            """Drain outT, reciprocate denominator row; returns obt pair."""
            obts = []
            for lb in range(2):
                obt = pobt.tile([EV, LB], F32, tag="obt", name=f"obt{lb}")
                nc.vector.tensor_copy(obt, pos[lb])
                nc.vector.reciprocal(obt[E : E + 1, :], obt[E : E + 1, :])
                obts.append(obt)
            return obts

        def emit_out(h, obts):
            osb = posb.tile([P, NLT, E], F32, tag="osb")
            for lt in range(NLT):
                obt = obts[lt // 4]
                pot = ppx.tile([P, EV], F32, tag="px")
                nc.tensor.transpose(
                    pot, obt[:, (lt % 4) * P : (lt % 4 + 1) * P],
                    identf[0:EV, 0:EV])
                nc.vector.tensor_scalar_mul(
                    osb[:, lt, :], pot[:, 0:E], pot[:, E : E + 1])
            nc.sync.dma_start(out=o_r[:, :, h * E : (h + 1) * E], in_=osb)

        prev_groups = None   # at-group tiles of head h-1
        prev_obts = None     # obt pair of head h-2
        for h in range(H + 1):
            pos = None
            if h >= 1:
                pos = [ppo.tile([EV, LB], F32, tag="po", name=f"po{h}")
                       for _ in range(2)]
            cur_groups = []
            for g in range(KGRP):
                if h < H:
                    cur_groups.append(emit_st_group(h, g))
                if h >= 1:
                    emit_av_group(h - 1, g, prev_groups[g], pos)
            obts = emit_av_end(pos) if h >= 1 else None
            if h >= 2:
                emit_out(h - 2, prev_obts)
            prev_groups, prev_obts = cur_groups, obts
        emit_out(H - 1, prev_obts)

    return nc


_nc_cache = None


def kernel(queries, keys, values, attn_mask=None, directional_weights=None,
           dynamic_param=None, **_unused):
    global _nc_cache, _last_exec_time_ns
    q = np.asarray(queries, dtype=np.float32).astype(ml_dtypes.bfloat16)
    k = np.asarray(keys, dtype=np.float32).astype(ml_dtypes.bfloat16)
    v = np.asarray(values, dtype=np.float32).astype(ml_dtypes.bfloat16)
    dw = np.asarray(directional_weights, dtype=np.float32).reshape(1, 1)
    dp = np.asarray(dynamic_param, dtype=np.float32).reshape(1, 1)

    if _nc_cache is None:
        nc = build_nc()
        nc.finalize()
        _nc_cache = nc
    nc = _nc_cache

    in_maps = []
    for c in range(8):
        b, lh = c // 2, c % 2
        in_maps.append({
            "q": np.ascontiguousarray(q[b, lh * LC : (lh + 1) * LC]).reshape(LC, D),
            "k": np.ascontiguousarray(k[b]).reshape(S, D),
            "v": np.ascontiguousarray(v[b]).reshape(S, D),
            "dw": dw, "dp": dp,
        })

    tracing = bool(os.environ.get("BASS_TRACE"))
    if tracing:
        _ensure_axon_hooks()
        import concourse.bass_utils as _bu

        _orig_upload = _bu.upload_artifacts
        _bu.upload_artifacts = lambda d: d  # no bucket access in this sandbox
        try:
            res = run_bass_kernel_spmd(nc, in_maps, core_ids=list(range(8)))
        except Exception as e:  # fall back to an untraced run
            print(f"traced run failed ({e!r}); retrying untraced", file=sys.stderr)
            os.environ["BASS_NEVER_TRACE"] = "1"
            try:
                res = run_bass_kernel_spmd(nc, in_maps, core_ids=list(range(8)))
            finally:
                os.environ.pop("BASS_NEVER_TRACE", None)
        finally:
            _bu.upload_artifacts = _orig_upload
    else:
        res = run_bass_kernel_spmd(nc, in_maps, core_ids=list(range(8)))
    _last_exec_time_ns = res.exec_time_ns

    out = np.empty((B, L, H, E), dtype=np.float32)
    for c in range(8):
        b, lh = c // 2, c % 2
        out[b, lh * LC : (lh + 1) * LC] = res.results[c]["o"].reshape(LC, H, E)
    return out


# revision 21
# speedup vs baseline: 1.1897x; 1.1897x over previous
"""Dynamic Directional Attention on 8 trn2 NeuronCores (Bass/Tile), v2.

Problem: B=4, L=S=2048, H=8, E=64, f32.
  qt = tanh(q * 1/(std_H(q)+eps) * dw) * dyn     (std over the HEAD dim, ddof=1)
  kt likewise; scores[b,h,l,s] = qt . kt          (contract E)
  tau[l] = sqrt(var_s(scores[l,:], ddof=1) + eps)
  A = softmax(scale * scores / tau);  out = A @ v  [B,L,H,E]

Sharding: 8 cores = 4 batches x 2 L-halves. Each core: q[b, half] = [1024, 512],
full k/v[b] = [2048, 512]. No collectives.

v2 changes vs baseline (375us):
  - q/k/v pre-cast to bf16 on host: DMA traffic halved, no input casts on chip.
  - A@V restructured V-stationary: lhsT = Vaug [s,65], rhs = A^T streaming
    N=512. Kills the 1024x ~110ns 128-col LDWEIGHTS the A-stationary layout
    paid (~110us of pure weight-load on the PE).
  - Output produced as outT [65, l] per head (row 64 = softmax denominator),
    denominator row reciprocated in place, PE-transposed back to [l, 65],
    scaled per-partition on DVE.
  - Transform stats batched: one Rsqrt activation over all 24 chunks' vars;
    table sets: rsqrt -> exp_and_others(tanh) -> rsqrt(m) -> exp (4 loads,
    vs 8 in the baseline).
  - ksum matmuls merged into the Gram matmuls via an ones column in the
    transformed-k tiles (pair-block layout [128, 2, 4, 130]).
  - PSUM->SBUF row-stat drains via DMA instead of DVE copies.
"""

import os
import sys

for _p in ("/opt/trn_rl_repo", "/root/.axon_site/_ro/trn_rl_repo"):
    if os.path.isdir(_p) and _p not in sys.path:
        sys.path.append(_p)

import numpy as np
import ml_dtypes

import concourse.bass as bass
import concourse.mybir as mybir
import concourse.tile as tile
from concourse import bacc
from concourse.bass_utils import run_bass_kernel_spmd
from concourse.masks import make_identity

F32 = mybir.dt.float32
BF16 = mybir.dt.bfloat16
AF = mybir.ActivationFunctionType
ALU = mybir.AluOpType

B, L, S, H, E = 4, 2048, 2048, 8, 64
LC = L // 2          # 1024 l-rows per core
D = H * E            # 512 free-dim columns per core
P = 128
NLT = LC // P        # 8 l-chunks
NST = S // P         # 16 s-chunks
LB = 512
NHP = H // 2         # 4 head-pairs
EV = E + 1           # 65: V block incl ones column
BLK = 130            # tn pair-block stride: 2*64 e-cols + ones + pad
NKG = 4              # at kk-group size
KGRP = NST // NKG    # 4 groups per head
EPS = 1e-6
SCALE = 1.0 / np.sqrt(E)
UNB_H = float(H) / float(H - 1)
UNB_S = float(S) / float(S - 1)

_last_exec_time_ns = None


def _ensure_axon_hooks():
    """Provide antenv.axon_hooks (NTFF profiling hook) if the image lacks it."""
    try:
        import antenv.axon_hooks  # noqa: F401

        return
    except ImportError:
        pass
    import contextlib
    import ctypes
    import types

    try:
        import antenv
    except ImportError:
        return

    holder = {"h": None}
    mod = types.ModuleType("antenv.axon_hooks")
    mod.set_axon_ntff_profile_hook = lambda h: holder.__setitem__("h", h)
    mod.get_axon_ntff_profile_hook = lambda: holder["h"]
    sys.modules["antenv.axon_hooks"] = mod
    antenv.axon_hooks = mod

    so_path = "/opt/axon/libaxon_pjrt.so"
    if not os.path.exists(so_path):
        return
    try:
        lib = ctypes.CDLL(so_path)
    except OSError:
        return
    if not hasattr(lib, "axon_start_nrt_profile"):
        return
    lib.axon_start_nrt_profile.argtypes = [
        ctypes.POINTER(ctypes.c_int64),
        ctypes.c_size_t,
    ]
    lib.axon_start_nrt_profile.restype = ctypes.c_int64
    lib.axon_stop_nrt_profile.argtypes = [ctypes.c_char_p]
    lib.axon_stop_nrt_profile.restype = ctypes.c_int64

    @contextlib.contextmanager
    def _hook(output_dir, device_ids):
        import jax

        jax.devices()
        if device_ids:
            ids = (ctypes.c_int64 * len(device_ids))(*device_ids)
            rc = lib.axon_start_nrt_profile(ids, len(device_ids))
        else:
            rc = lib.axon_start_nrt_profile(None, 0)
        if rc != 0:
            raise RuntimeError(f"axon_start_nrt_profile rc={rc}")
        try:
            yield
        finally:
            n = lib.axon_stop_nrt_profile(str(output_dir).encode())
            print(f"profile: {n} file(s) written to {output_dir}", file=sys.stderr)

    holder["h"] = _hook


def _head_bcast(ap_2d, nh=H, ne=E):
    """View a [p, ne] AP as [p, nh, ne] with the head dim broadcast (step 0)."""
    return bass.AP(
        tensor=ap_2d.tensor,
        offset=ap_2d.offset,
        ap=[list(ap_2d.ap[0]), [0, nh], list(ap_2d.ap[1])],
    )


def build_nc():
    nc = bacc.Bacc("TRN2", target_bir_lowering=False, debug=False)
    q_d = nc.dram_tensor("q", [LC, D], BF16, kind="ExternalInput")
    k_d = nc.dram_tensor("k", [S, D], BF16, kind="ExternalInput")
    v_d = nc.dram_tensor("v", [S, D], BF16, kind="ExternalInput")
    dw_d = nc.dram_tensor("dw", [1, 1], F32, kind="ExternalInput")
    dp_d = nc.dram_tensor("dp", [1, 1], F32, kind="ExternalInput")
    o_d = nc.dram_tensor("o", [LC, D], F32, kind="ExternalOutput")

    q_r = q_d.rearrange("(n p) d -> p n d", p=P)
    k_r = k_d.rearrange("(n p) d -> p n d", p=P)
    v_r = v_d.rearrange("(n p) d -> p n d", p=P)
    o_r = o_d.rearrange("(n p) d -> p n d", p=P)

    from contextlib import ExitStack

    with tile.TileContext(nc) as tc, ExitStack() as ctx:
        ek = ctx.enter_context
        sing = ek(tc.tile_pool(name="sing", bufs=1))
        pnat = ek(tc.tile_pool(name="nat", bufs=12))
        psq = ek(tc.tile_pool(name="sq", bufs=2))
        pstat = ek(tc.tile_pool(name="stat", bufs=1))
        ptn = ek(tc.tile_pool(name="tn", bufs=12))
        pqt = ek(tc.tile_pool(name="qt", bufs=4))
        pgw = ek(tc.tile_pool(name="gw", bufs=2))
        prw = ek(tc.tile_pool(name="rows", bufs=1))
        pdr = ek(tc.tile_pool(name="dr", bufs=1, space="DRAM"))
        pmb = ek(tc.tile_pool(name="mb", bufs=2))
        pqts = ek(tc.tile_pool(name="qts", bufs=4))
        pat = ek(tc.tile_pool(name="at", bufs=5))          # kk-group A^T bf16
        pva = ek(tc.tile_pool(name="va", bufs=1))
        pvn = ek(tc.tile_pool(name="vn", bufs=2))
        pobt = ek(tc.tile_pool(name="obt", bufs=4))        # [65,512] f32 outT
        posb = ek(tc.tile_pool(name="osb", bufs=2))
        pps = ek(tc.tile_pool(name="ps", bufs=2, space="PSUM"))    # [128,1024]
        ppo = ek(tc.tile_pool(name="po", bufs=2, space="PSUM"))    # [65,512]
        ppx = ek(tc.tile_pool(name="px", bufs=2, space="PSUM"))    # shared 2KB

        # --- constants ---
        ident = sing.tile([P, P], BF16)
        make_identity(nc, ident)
        identf = sing.tile([P, P], F32)
        make_identity(nc, identf)
        zero_t = sing.tile([P, 1], F32)
        nc.vector.memset(zero_t, 0.0)
        eps_t = sing.tile([P, 1], F32)
        nc.vector.memset(eps_t, EPS)
        dw_t = sing.tile([P, 1], F32)
        nc.sync.dma_start(out=dw_t, in_=dw_d[:, :].to_broadcast([P, 1]))
        dp_t = sing.tile([P, 1], F32)
        nc.sync.dma_start(out=dp_t, in_=dp_d[:, :].to_broadcast([P, 1]))
        dp2 = sing.tile([P, 1], F32)
        nc.vector.tensor_mul(dp2, dp_t, dp_t)
        c2 = sing.tile([P, 1], F32)  # scale * dyn^2
        nc.vector.tensor_scalar_mul(c2, dp2, float(SCALE))
        dp4 = sing.tile([P, 1], F32)
        nc.vector.tensor_mul(dp4, dp2, dp2)
        a_t = sing.tile([P, 1], F32)  # dyn^4 * UNB_S / S
        nc.vector.tensor_scalar_mul(a_t, dp4, UNB_S / S)
        b_t = sing.tile([P, 1], F32)  # dyn^4 * UNB_S / S^2
        nc.vector.tensor_scalar_mul(b_t, dp4, UNB_S / S / S)
        o2 = sing.tile([P, 2], BF16)  # per-head ones blocks for row sums
        nc.vector.memset(o2, 0.0)
        nc.vector.memset(o2[0:E, 0:1], 1.0)
        nc.vector.memset(o2[E:P, 1:2], 1.0)

        # --- V path: load v (bf16), pack Vaug [128, kk, h, 65] ---
        va = pva.tile([P, NST, H, EV], BF16)
        nc.vector.memset(
            va.rearrange("p a h c -> p (a h) c")[:, :, E : E + 1], 1.0)
        for kk in range(NST):
            vn = pvn.tile([P, D], BF16, tag="vn")
            nc.sync.dma_start(out=vn, in_=v_r[:, kk, :])
            nc.vector.tensor_copy(
                va[:, kk, :, 0:E],
                vn.rearrange("p (h e) -> p h e", h=H))

        # --- T1: load q/k nat pairs; per-chunk head sums into batched tiles ---
        NPAIR = (NLT + NST) // 2  # 12
        nat_pairs = []
        ssum_all = pstat.tile([P, NPAIR, 2, E], F32, tag="ssum")
        ssq_all = pstat.tile([P, NPAIR, 2, E], F32, tag="ssq")
        rstd_all = ssum_all  # reused once mean^2 is consumed by the var stt

        def t1_pair(src_r, i0, pi):
            nat = pnat.tile([P, 2, D], BF16, tag="nat")
            for u in range(2):
                nc.sync.dma_start(out=nat[:, u, :], in_=src_r[:, i0 + u, :])
            for u in range(2):
                sq = psq.tile([P, D], F32, tag="sq")
                nc.gpsimd.tensor_mul(sq, nat[:, u, :], nat[:, u, :])
                nc.vector.tensor_reduce(
                    ssum_all[:, pi, u, :],
                    nat[:, u, :].rearrange("p (h e) -> p e h", h=H),
                    axis=mybir.AxisListType.X, op=ALU.add)
                nc.vector.tensor_reduce(
                    ssq_all[:, pi, u, :],
                    sq.rearrange("p (h e) -> p e h", h=H),
                    axis=mybir.AxisListType.X, op=ALU.add)
            nat_pairs.append(nat)

        for j in range(NLT // 2):
            t1_pair(q_r, 2 * j, j)
        for j in range(NST // 2):
            t1_pair(k_r, 2 * j, NLT // 2 + j)

        # batched: mean^2 and biased var; rstd = rsqrt(UNB_H*var + eps)
        flat_sum = ssum_all.rearrange("p a b e -> p (a b e)")
        flat_sq = ssq_all.rearrange("p a b e -> p (a b e)")
        nc.vector.tensor_scalar_mul(flat_sum, flat_sum, 1.0 / H)
        nc.vector.tensor_mul(flat_sum, flat_sum, flat_sum)
        nc.vector.scalar_tensor_tensor(
            flat_sq, flat_sq, 1.0 / H, flat_sum,
            op0=ALU.mult, op1=ALU.subtract)
        # rstd = sqrt(1/(UNB_H*var + eps)) via fast DVE reciprocal + ACT Sqrt
        nc.vector.tensor_scalar(
            flat_sq, flat_sq, UNB_H, EPS, op0=ALU.mult, op1=ALU.add)
        flat_rstd = rstd_all.rearrange("p a b e -> p (a b e)")
        nc.vector.reciprocal_approx_fast(flat_rstd, flat_sq)
        nc.scalar.activation(flat_rstd, flat_rstd, AF.Sqrt,
                             bias=zero_t, scale=1.0)

        # --- T3: tmp = nat*rstd(bcast heads); tanh -> tn [128,2,4,130] ---
        tn_pairs = []
        for pi, nat in enumerate(nat_pairs):
            tn = ptn.tile([P, 2, NHP, BLK], BF16, tag="tn")
            for u in range(2):
                tmp = psq.tile([P, H, E], BF16, tag="tmp")
                nc.vector.tensor_mul(
                    tmp, nat[:, u, :].rearrange("p (h e) -> p h e", h=H),
                    _head_bcast(rstd_all[:, pi, u, :]))
                nc.scalar.activation(
                    tn[:, u, :, 0 : 2 * E], tmp.rearrange("p h e -> p (h e)"),
                    AF.Tanh, bias=zero_t, scale=dw_t)
            tn_pairs.append(tn)
        tn_q = tn_pairs[: NLT // 2]
        tn_k = tn_pairs[NLT // 2 :]
        for tn in tn_k:  # ones column feeding ksum via the Gram matmul
            nc.vector.memset(
                tn.rearrange("p u h c -> p (u h) c")[:, :, 2 * E : 2 * E + 1],
                1.0)

        # --- TR: PE-transpose tq/tk into [2E, l]/[2E, s] per pair ---
        tqT = [pqt.tile([P, LC], BF16, tag="tqT", name=f"tqT{i}")
               for i in range(NHP)]
        tkT = [pqt.tile([P, S], BF16, tag="tkT", name=f"tkT{i}")
               for i in range(NHP)]
        for tiles, dsts, npair in ((tn_q, tqT, NLT // 2), (tn_k, tkT, NST // 2)):
            for j in range(0, npair, 2):
                for hp in range(NHP):
                    pt = ppx.tile([P, 4, P], BF16, tag="px")
                    for w in range(2):
                        for u in range(2):
                            nc.tensor.transpose(
                                pt[:, 2 * w + u, :],
                                tiles[j + w][:, u, hp, 0 : 2 * E], ident)
                    nc.vector.tensor_copy(
                        dsts[hp][:, (2 * j) * P : (2 * j + 4) * P],
                        pt.rearrange("p a b -> p (a b)"))

        # --- G phase per pair: block-diag Gram + ksum column; row stats ---
        ssq_sb = prw.tile([8, LC], F32, tag="ssqsb")
        rsum_sb = prw.tile([8, LC], F32, tag="rsumsb")
        for hp in range(NHP):
            g_ps = ppx.tile([P, 2 * E + 1], F32, tag="px")
            for idx in range(NST):
                tn = tn_k[idx // 2]
                u = idx % 2
                nc.tensor.matmul(
                    g_ps, tn[:, u, hp, 0 : 2 * E], tn[:, u, hp, 0 : 2 * E + 1],
                    start=(idx == 0), stop=(idx == NST - 1))
            gsb = pgw.tile([P, P], BF16, tag="gsb")
            nc.vector.tensor_copy(gsb, g_ps[:, 0 : 2 * E])
            nc.vector.memset(gsb[0:E, E:P], 0.0)
            nc.vector.memset(gsb[E:P, 0:E], 0.0)
            k2 = pgw.tile([P, 2], BF16, tag="k2")
            nc.vector.memset(k2, 0.0)
            nc.vector.tensor_copy(k2[0:E, 0:1], g_ps[0:E, 2 * E : 2 * E + 1])
            nc.vector.tensor_copy(k2[E:P, 1:2], g_ps[E:P, 2 * E : 2 * E + 1])

            wps = pps.tile([P, LC], F32, tag="ps")
            for j in range(2):
                nc.tensor.matmul(wps[:, j * LB : (j + 1) * LB], gsb,
                                 tqT[hp][:, j * LB : (j + 1) * LB],
                                 start=True, stop=True)
            wsb = pgw.tile([P, LC], BF16, tag="wsb")
            nc.vector.tensor_copy(wsb, wps)
            prod = pgw.tile([P, LC], BF16, tag="prod")
            nc.vector.tensor_mul(prod, tqT[hp], wsb)
            stg_ss = pgw.tile([2, LC], F32, tag="stgss", bufs=1)
            stg_rs = pgw.tile([2, LC], F32, tag="stgrs", bufs=1)
            for j in range(2):
                r_ss = ppx.tile([2, LB], F32, tag="px")
                nc.tensor.matmul(r_ss, o2, prod[:, j * LB : (j + 1) * LB],
                                 start=True, stop=True)
                nc.vector.tensor_copy(stg_ss[:, j * LB : (j + 1) * LB], r_ss)
                r_rs = ppx.tile([2, LB], F32, tag="px")
                nc.tensor.matmul(r_rs, k2, tqT[hp][:, j * LB : (j + 1) * LB],
                                 start=True, stop=True)
                nc.vector.tensor_copy(stg_rs[:, j * LB : (j + 1) * LB], r_rs)
            nc.sync.dma_start(out=ssq_sb[2 * hp : 2 * hp + 2, :], in_=stg_ss)
            nc.sync.dma_start(out=rsum_sb[2 * hp : 2 * hp + 2, :], in_=stg_rs)

        # m = c2 * rsqrt(ssq*a - rsum^2*b + eps), vectorized over 8 heads
        nc.vector.tensor_mul(rsum_sb, rsum_sb, rsum_sb)
        nc.vector.tensor_scalar_mul(rsum_sb, rsum_sb, b_t[0:8, :])
        nc.vector.scalar_tensor_tensor(
            ssq_sb, ssq_sb, a_t[0:8, :], rsum_sb,
            op0=ALU.mult, op1=ALU.subtract)
        # m = c2 / sqrt(tau2 + eps): fast reciprocal then Sqrt
        nc.vector.tensor_scalar_add(ssq_sb, ssq_sb, EPS)
        nc.vector.reciprocal_approx_fast(rsum_sb, ssq_sb)
        nc.scalar.activation(rsum_sb, rsum_sb, AF.Sqrt,
                             bias=zero_t[0:8, :], scale=1.0)
        nc.vector.tensor_scalar_mul(rsum_sb, rsum_sb, c2[0:8, :])
        m8b = prw.tile([8, LC], BF16, tag="m8b")
        nc.vector.tensor_copy(m8b, rsum_sb)
        mdr = pdr.tile([8, LC], BF16, tag="mdr")
        nc.sync.dma_start(out=mdr[:, :], in_=m8b)

        # qts per pair: tq * m (broadcast m rows from DRAM)
        qts_l = []
        for hp in range(NHP):
            mb = pmb.tile([P, LC], BF16, tag="mb")
            for local in range(2):
                h = 2 * hp + local
                nc.sync.dma_start(out=mb[local * E : (local + 1) * E, :],
                                  in_=mdr[h : h + 1, :].to_broadcast([E, LC]))
            qts = pqts.tile([P, LC], BF16, tag="qts")
            nc.vector.tensor_mul(qts, tqT[hp], mb)
            qts_l.append(qts)

        # --- head loop: st -> exp -> V-stationary A@V -> untranspose,
        #     interleaved at kk-group granularity so PE alternates between
        #     st(h) and AV(h-1) while ACT streams exp(h) ---
        def emit_st_group(h, g):
            """st + exp for head h, kk group g; returns the at-group tile."""
            hp, local = h // 2, h % 2
            off = local * E
            tk = tkT[hp]
            at = pat.tile([P, NKG, LC], BF16, tag="at")
            for kg in range(NKG):
                kk = g * NKG + kg
                st_ps = pps.tile([P, LC], F32, tag="ps")
                for lb in range(2):
                    nc.tensor.matmul(
                        st_ps[:, lb * LB : (lb + 1) * LB],
                        tk[off : off + E, kk * P : (kk + 1) * P],
                        qts_l[hp][off : off + E, lb * LB : (lb + 1) * LB],
                        start=True, stop=True)
                nc.scalar.activation(at[:, kg, :], st_ps, AF.Exp,
                                     bias=zero_t, scale=1.0)
            return at

        def emit_av_group(h, g, at, pos):
            for kg in range(NKG):
                kk = g * NKG + kg
                for lb in range(2):
                    nc.tensor.matmul(
                        pos[lb], va[:, kk, h, :],
                        at[:, kg, lb * LB : (lb + 1) * LB],
                        start=(kk == 0), stop=(kk == NST - 1))

        def emit_av_end(pos):
            """Drain outT, reciprocate denominator row; returns obt pair."""
            obts = []
            for lb in range(2):
                obt = pobt.tile([EV, LB], F32, tag="obt", name=f"obt{lb}")
                nc.vector.tensor_copy(obt, pos[lb])
                obts.append(obt)
            return obts

        def emit_out(h, obts):
            osb = posb.tile([P, NLT, E], F32, tag="osb")
            den = posb.tile([P, NLT], F32, tag="den")
            for lt in range(NLT):
                obt = obts[lt // 4]
                pot = ppx.tile([P, EV], F32, tag="px")
                nc.tensor.transpose(
                    pot, obt[:, (lt % 4) * P : (lt % 4 + 1) * P],
                    identf[0:EV, 0:EV])
                nc.vector.tensor_copy(osb[:, lt, :], pot[:, 0:E])
                nc.vector.tensor_copy(den[:, lt : lt + 1], pot[:, E : E + 1])
            nc.vector.reciprocal(den, den)
            den_b = bass.AP(
                tensor=den.tensor, offset=den.offset,
                ap=[list(den.ap[0]), list(den.ap[1]), [0, E]])
            nc.vector.tensor_mul(
                osb.rearrange("p n e -> p n e"), osb, den_b)
            nc.sync.dma_start(out=o_r[:, :, h * E : (h + 1) * E], in_=osb)

        # Per head: first AV(h-1) as one dense 32-matmul PE burst (keeps the
        # HAM clock-gate warm), then the ACT-paced st+exp phase for head h.
        prev_groups = None   # at-group tiles of head h-1
        prev_obts = None     # obt pair of head h-2
        for h in range(H + 1):
            obts = None
            if h >= 1:
                pos = [ppo.tile([EV, LB], F32, tag="po", name=f"po{h}")
                       for _ in range(2)]
                for g in range(KGRP):
                    emit_av_group(h - 1, g, prev_groups[g], pos)
                obts = emit_av_end(pos)
            cur_groups = []
            if h < H:
                for g in range(KGRP):
                    cur_groups.append(emit_st_group(h, g))
            if h >= 2:
                emit_out(h - 2, prev_obts)
            prev_groups, prev_obts = cur_groups, obts
        emit_out(H - 1, prev_obts)

    return nc


_nc_cache = None


def kernel(queries, keys, values, attn_mask=None, directional_weights=None,
           dynamic_param=None, **_unused):
    global _nc_cache, _last_exec_time_ns
    q = np.asarray(queries, dtype=np.float32).astype(ml_dtypes.bfloat16)
    k = np.asarray(keys, dtype=np.float32).astype(ml_dtypes.bfloat16)
    v = np.asarray(values, dtype=np.float32).astype(ml_dtypes.bfloat16)
    dw = np.asarray(directional_weights, dtype=np.float32).reshape(1, 1)
    dp = np.asarray(dynamic_param, dtype=np.float32).reshape(1, 1)

    if _nc_cache is None:
        nc = build_nc()
        nc.finalize()
        _nc_cache = nc
    nc = _nc_cache

    in_maps = []
    for c in range(8):
        b, lh = c // 2, c % 2
        in_maps.append({
            "q": np.ascontiguousarray(q[b, lh * LC : (lh + 1) * LC]).reshape(LC, D),
            "k": np.ascontiguousarray(k[b]).reshape(S, D),
            "v": np.ascontiguousarray(v[b]).reshape(S, D),
            "dw": dw, "dp": dp,
        })

    tracing = bool(os.environ.get("BASS_TRACE"))
    if tracing:
        _ensure_axon_hooks()
        import concourse.bass_utils as _bu

        _orig_upload = _bu.upload_artifacts
        _bu.upload_artifacts = lambda d: d  # no bucket access in this sandbox
        try:
            res = run_bass_kernel_spmd(nc, in_maps, core_ids=list(range(8)))
        except Exception as e:  # fall back to an untraced run
            print(f"traced run failed ({e!r}); retrying untraced", file=sys.stderr)
            os.environ["BASS_NEVER_TRACE"] = "1"
            try:
                res = run_bass_kernel_spmd(nc, in_maps, core_ids=list(range(8)))
            finally:
                os.environ.pop("BASS_NEVER_TRACE", None)
        finally:
            _bu.upload_artifacts = _orig_upload
    else:
        res = run_bass_kernel_spmd(nc, in_maps, core_ids=list(range(8)))
    _last_exec_time_ns = res.exec_time_ns

    out = np.empty((B, L, H, E), dtype=np.float32)
    for c in range(8):
        b, lh = c // 2, c % 2
        out[b, lh * LC : (lh + 1) * LC] = res.results[c]["o"].reshape(LC, H, E)
    return out


# revision 26
# speedup vs baseline: 1.3350x; 1.1221x over previous
"""Dynamic Directional Attention on 8 trn2 NeuronCores (Bass/Tile), v2.

Problem: B=4, L=S=2048, H=8, E=64, f32.
  qt = tanh(q * 1/(std_H(q)+eps) * dw) * dyn     (std over the HEAD dim, ddof=1)
  kt likewise; scores[b,h,l,s] = qt . kt          (contract E)
  tau[l] = sqrt(var_s(scores[l,:], ddof=1) + eps)
  A = softmax(scale * scores / tau);  out = A @ v  [B,L,H,E]

Sharding: 8 cores = 4 batches x 2 L-halves. Each core: q[b, half] = [1024, 512],
full k/v[b] = [2048, 512]. No collectives.

v2 changes vs baseline (375us):
  - q/k/v pre-cast to bf16 on host: DMA traffic halved, no input casts on chip.
  - A@V restructured V-stationary: lhsT = Vaug [s,65], rhs = A^T streaming
    N=512. Kills the 1024x ~110ns 128-col LDWEIGHTS the A-stationary layout
    paid (~110us of pure weight-load on the PE).
  - Output produced as outT [65, l] per head (row 64 = softmax denominator),
    denominator row reciprocated in place, PE-transposed back to [l, 65],
    scaled per-partition on DVE.
  - Transform stats batched: one Rsqrt activation over all 24 chunks' vars;
    table sets: rsqrt -> exp_and_others(tanh) -> rsqrt(m) -> exp (4 loads,
    vs 8 in the baseline).
  - ksum matmuls merged into the Gram matmuls via an ones column in the
    transformed-k tiles (pair-block layout [128, 2, 4, 130]).
  - PSUM->SBUF row-stat drains via DMA instead of DVE copies.
"""

import os
import sys

for _p in ("/opt/trn_rl_repo", "/root/.axon_site/_ro/trn_rl_repo"):
    if os.path.isdir(_p) and _p not in sys.path:
        sys.path.append(_p)

import numpy as np
import ml_dtypes

import concourse.bass as bass
import concourse.mybir as mybir
import concourse.tile as tile
from concourse import bacc
from concourse.bass_utils import run_bass_kernel_spmd
from concourse.masks import make_identity

F32 = mybir.dt.float32
BF16 = mybir.dt.bfloat16
AF = mybir.ActivationFunctionType
ALU = mybir.AluOpType

B, L, S, H, E = 4, 2048, 2048, 8, 64
LC = L // 2          # 1024 l-rows per core
D = H * E            # 512 free-dim columns per core
P = 128
NLT = LC // P        # 8 l-chunks
NST = S // P         # 16 s-chunks
LB = 512
NHP = H // 2         # 4 head-pairs
EV = E + 1           # 65: V block incl ones column
BLK = 130            # tn pair-block stride: 2*64 e-cols + ones + pad
NKG = 4              # at kk-group size
KGRP = NST // NKG    # 4 groups per head
EPS = 1e-6
SCALE = 1.0 / np.sqrt(E)
UNB_H = float(H) / float(H - 1)
UNB_S = float(S) / float(S - 1)

_last_exec_time_ns = None


def _ensure_axon_hooks():
    """Provide antenv.axon_hooks (NTFF profiling hook) if the image lacks it."""
    try:
        import antenv.axon_hooks  # noqa: F401

        return
    except ImportError:
        pass
    import contextlib
    import ctypes
    import types

    try:
        import antenv
    except ImportError:
        return

    holder = {"h": None}
    mod = types.ModuleType("antenv.axon_hooks")
    mod.set_axon_ntff_profile_hook = lambda h: holder.__setitem__("h", h)
    mod.get_axon_ntff_profile_hook = lambda: holder["h"]
    sys.modules["antenv.axon_hooks"] = mod
    antenv.axon_hooks = mod

    so_path = "/opt/axon/libaxon_pjrt.so"
    if not os.path.exists(so_path):
        return
    try:
        lib = ctypes.CDLL(so_path)
    except OSError:
        return
    if not hasattr(lib, "axon_start_nrt_profile"):
        return
    lib.axon_start_nrt_profile.argtypes = [
        ctypes.POINTER(ctypes.c_int64),
        ctypes.c_size_t,
    ]
    lib.axon_start_nrt_profile.restype = ctypes.c_int64
    lib.axon_stop_nrt_profile.argtypes = [ctypes.c_char_p]
    lib.axon_stop_nrt_profile.restype = ctypes.c_int64

    @contextlib.contextmanager
    def _hook(output_dir, device_ids):
        import jax

        jax.devices()
        if device_ids:
            ids = (ctypes.c_int64 * len(device_ids))(*device_ids)
            rc = lib.axon_start_nrt_profile(ids, len(device_ids))
        else:
            rc = lib.axon_start_nrt_profile(None, 0)
        if rc != 0:
            raise RuntimeError(f"axon_start_nrt_profile rc={rc}")
        try:
            yield
        finally:
            n = lib.axon_stop_nrt_profile(str(output_dir).encode())
            print(f"profile: {n} file(s) written to {output_dir}", file=sys.stderr)

    holder["h"] = _hook


def _head_bcast(ap_2d, nh=H, ne=E):
    """View a [p, ne] AP as [p, nh, ne] with the head dim broadcast (step 0)."""
    return bass.AP(
        tensor=ap_2d.tensor,
        offset=ap_2d.offset,
        ap=[list(ap_2d.ap[0]), [0, nh], list(ap_2d.ap[1])],
    )


def build_nc():
    nc = bacc.Bacc("TRN2", target_bir_lowering=False, debug=False)
    q_d = nc.dram_tensor("q", [LC, D], BF16, kind="ExternalInput")
    k_d = nc.dram_tensor("k", [S, D], BF16, kind="ExternalInput")
    v_d = nc.dram_tensor("v", [S, D], BF16, kind="ExternalInput")
    dw_d = nc.dram_tensor("dw", [1, 1], F32, kind="ExternalInput")
    dp_d = nc.dram_tensor("dp", [1, 1], F32, kind="ExternalInput")
    o_d = nc.dram_tensor("o", [LC, D], F32, kind="ExternalOutput")

    q_r = q_d.rearrange("(n p) d -> p n d", p=P)
    k_r = k_d.rearrange("(n p) d -> p n d", p=P)
    v_r = v_d.rearrange("(n p) d -> p n d", p=P)
    o_r = o_d.rearrange("(n p) d -> p n d", p=P)

    from contextlib import ExitStack

    with tile.TileContext(nc) as tc, ExitStack() as ctx:
        ek = ctx.enter_context
        sing = ek(tc.tile_pool(name="sing", bufs=1))
        pnat = ek(tc.tile_pool(name="nat", bufs=12))
        psq = ek(tc.tile_pool(name="sq", bufs=2))
        pstat = ek(tc.tile_pool(name="stat", bufs=1))
        ptn = ek(tc.tile_pool(name="tn", bufs=12))
        pqt = ek(tc.tile_pool(name="qt", bufs=4))
        pgw = ek(tc.tile_pool(name="gw", bufs=2))
        prw = ek(tc.tile_pool(name="rows", bufs=1))
        pdr = ek(tc.tile_pool(name="dr", bufs=1, space="DRAM"))
        pmb = ek(tc.tile_pool(name="mb", bufs=2))
        pqts = ek(tc.tile_pool(name="qts", bufs=4))
        pat = ek(tc.tile_pool(name="at", bufs=5))          # kk-group A^T bf16
        pva = ek(tc.tile_pool(name="va", bufs=1))
        pvn = ek(tc.tile_pool(name="vn", bufs=2))
        pobt = ek(tc.tile_pool(name="obt", bufs=4))        # [65,512] f32 outT
        posb = ek(tc.tile_pool(name="osb", bufs=2))
        pps = ek(tc.tile_pool(name="ps", bufs=2, space="PSUM"))    # [128,1024]
        ppo = ek(tc.tile_pool(name="po", bufs=2, space="PSUM"))    # [65,512]
        ppx = ek(tc.tile_pool(name="px", bufs=2, space="PSUM"))    # shared 2KB

        # --- constants ---
        ident = sing.tile([P, P], BF16)
        make_identity(nc, ident)
        identf = sing.tile([P, P], F32)
        make_identity(nc, identf)
        zero_t = sing.tile([P, 1], F32)
        nc.vector.memset(zero_t, 0.0)
        eps_t = sing.tile([P, 1], F32)
        nc.vector.memset(eps_t, EPS)
        dw_t = sing.tile([P, 1], F32)
        nc.sync.dma_start(out=dw_t, in_=dw_d[:, :].to_broadcast([P, 1]))
        dp_t = sing.tile([P, 1], F32)
        nc.sync.dma_start(out=dp_t, in_=dp_d[:, :].to_broadcast([P, 1]))
        dp2 = sing.tile([P, 1], F32)
        nc.vector.tensor_mul(dp2, dp_t, dp_t)
        c2 = sing.tile([P, 1], F32)  # scale * dyn^2
        nc.vector.tensor_scalar_mul(c2, dp2, float(SCALE))
        dp4 = sing.tile([P, 1], F32)
        nc.vector.tensor_mul(dp4, dp2, dp2)
        a_t = sing.tile([P, 1], F32)  # dyn^4 * UNB_S / S
        nc.vector.tensor_scalar_mul(a_t, dp4, UNB_S / S)
        b_t = sing.tile([P, 1], F32)  # dyn^4 * UNB_S / S^2
        nc.vector.tensor_scalar_mul(b_t, dp4, UNB_S / S / S)
        o2 = sing.tile([P, 2], BF16)  # per-head ones blocks for row sums
        nc.vector.memset(o2, 0.0)
        nc.vector.memset(o2[0:E, 0:1], 1.0)
        nc.vector.memset(o2[E:P, 1:2], 1.0)

        # --- V path: load v (bf16), pack Vaug [128, kk, h, 65] ---
        va = pva.tile([P, NST, H, EV], BF16)
        nc.vector.memset(
            va.rearrange("p a h c -> p (a h) c")[:, :, E : E + 1], 1.0)
        for kk in range(NST):
            vn = pvn.tile([P, D], BF16, tag="vn")
            nc.sync.dma_start(out=vn, in_=v_r[:, kk, :])
            nc.vector.tensor_copy(
                va[:, kk, :, 0:E],
                vn.rearrange("p (h e) -> p h e", h=H))

        # --- T1: load q/k nat pairs; per-chunk head sums into batched tiles ---
        NPAIR = (NLT + NST) // 2  # 12
        nat_pairs = []
        ssum_all = pstat.tile([P, NPAIR, 2, E], F32, tag="ssum")
        ssq_all = pstat.tile([P, NPAIR, 2, E], F32, tag="ssq")
        rstd_all = ssum_all  # reused once mean^2 is consumed by the var stt

        def halving_sum(src, dst, scratch):
            """dst[p,64] f32 = sum of 8 head-blocks of src[p,512] bf16."""
            s1 = scratch[:, 0:256]
            nc.vector.tensor_add(s1, src[:, 0:256], src[:, 256:512])
            s2 = scratch[:, 256:384]
            nc.vector.tensor_add(s2, s1[:, 0:128], s1[:, 128:256])
            nc.vector.tensor_add(dst, s2[:, 0:64], s2[:, 64:128])

        def t1_pair(src_r, i0, pi):
            nat = pnat.tile([P, 2, D], BF16, tag="nat")
            for u in range(2):
                nc.sync.dma_start(out=nat[:, u, :], in_=src_r[:, i0 + u, :])
            for u in range(2):
                sq = psq.tile([P, D], BF16, tag="sq")
                nc.gpsimd.tensor_mul(sq, nat[:, u, :], nat[:, u, :])
                hs = psq.tile([P, 384], BF16, tag="hs")
                halving_sum(nat[:, u, :], ssum_all[:, pi, u, :], hs)
                halving_sum(sq, ssq_all[:, pi, u, :], hs)
            nat_pairs.append(nat)

        for j in range(NLT // 2):
            t1_pair(q_r, 2 * j, j)
        for j in range(NST // 2):
            t1_pair(k_r, 2 * j, NLT // 2 + j)

        # batched: mean^2 and biased var; rstd = rsqrt(UNB_H*var + eps)
        flat_sum = ssum_all.rearrange("p a b e -> p (a b e)")
        flat_sq = ssq_all.rearrange("p a b e -> p (a b e)")
        nc.vector.tensor_scalar_mul(flat_sum, flat_sum, 1.0 / H)
        nc.vector.tensor_mul(flat_sum, flat_sum, flat_sum)
        nc.vector.scalar_tensor_tensor(
            flat_sq, flat_sq, 1.0 / H, flat_sum,
            op0=ALU.mult, op1=ALU.subtract)
        # rstd = sqrt(1/(UNB_H*var + eps)) via fast DVE reciprocal + ACT Sqrt
        nc.vector.tensor_scalar(
            flat_sq, flat_sq, UNB_H, EPS, op0=ALU.mult, op1=ALU.add)
        flat_rstd = rstd_all.rearrange("p a b e -> p (a b e)")
        nc.vector.reciprocal_approx_fast(flat_rstd, flat_sq)
        nc.scalar.activation(flat_rstd, flat_rstd, AF.Sqrt,
                             bias=zero_t, scale=1.0)

        # --- T3: tmp = nat*rstd(bcast heads); tanh -> tn [128,2,4,130] ---
        tn_pairs = []
        for pi, nat in enumerate(nat_pairs):
            tn = ptn.tile([P, 2, NHP, BLK], BF16, tag="tn")
            for u in range(2):
                tmp = psq.tile([P, H, E], BF16, tag="tmp")
                nc.gpsimd.tensor_mul(
                    tmp, nat[:, u, :].rearrange("p (h e) -> p h e", h=H),
                    _head_bcast(rstd_all[:, pi, u, :]))
                nc.scalar.activation(
                    tn[:, u, :, 0 : 2 * E], tmp.rearrange("p h e -> p (h e)"),
                    AF.Tanh, bias=zero_t, scale=dw_t)
            tn_pairs.append(tn)
        tn_q = tn_pairs[: NLT // 2]
        tn_k = tn_pairs[NLT // 2 :]
        for tn in tn_k:  # ones column feeding ksum via the Gram matmul
            nc.vector.memset(
                tn.rearrange("p u h c -> p (u h) c")[:, :, 2 * E : 2 * E + 1],
                1.0)

        # --- TR: PE-transpose tq/tk into [2E, l]/[2E, s] per pair ---
        tqT = [pqt.tile([P, LC], BF16, tag="tqT", name=f"tqT{i}")
               for i in range(NHP)]
        tkT = [pqt.tile([P, S], BF16, tag="tkT", name=f"tkT{i}")
               for i in range(NHP)]
        for tiles, dsts, npair in ((tn_q, tqT, NLT // 2), (tn_k, tkT, NST // 2)):
            for j in range(0, npair, 2):
                for hp in range(NHP):
                    pt = ppx.tile([P, 4, P], BF16, tag="px")
                    for w in range(2):
                        for u in range(2):
                            nc.tensor.transpose(
                                pt[:, 2 * w + u, :],
                                tiles[j + w][:, u, hp, 0 : 2 * E], ident)
                    nc.vector.tensor_copy(
                        dsts[hp][:, (2 * j) * P : (2 * j + 4) * P],
                        pt.rearrange("p a b -> p (a b)"))

        # --- G phase per pair: block-diag Gram + ksum column; row stats ---
        ssq_sb = prw.tile([8, LC], F32, tag="ssqsb")
        rsum_sb = prw.tile([8, LC], F32, tag="rsumsb")
        for hp in range(NHP):
            g_ps = ppx.tile([P, 2 * E + 1], F32, tag="px")
            for idx in range(NST):
                tn = tn_k[idx // 2]
                u = idx % 2
                nc.tensor.matmul(
                    g_ps, tn[:, u, hp, 0 : 2 * E], tn[:, u, hp, 0 : 2 * E + 1],
                    start=(idx == 0), stop=(idx == NST - 1))
            gsb = pgw.tile([P, P], BF16, tag="gsb")
            nc.vector.tensor_copy(gsb, g_ps[:, 0 : 2 * E])
            nc.vector.memset(gsb[0:E, E:P], 0.0)
            nc.vector.memset(gsb[E:P, 0:E], 0.0)
            k2 = pgw.tile([P, 2], BF16, tag="k2")
            nc.vector.memset(k2, 0.0)
            nc.vector.tensor_copy(k2[0:E, 0:1], g_ps[0:E, 2 * E : 2 * E + 1])
            nc.vector.tensor_copy(k2[E:P, 1:2], g_ps[E:P, 2 * E : 2 * E + 1])

            wps = pps.tile([P, LC], F32, tag="ps")
            for j in range(2):
                nc.tensor.matmul(wps[:, j * LB : (j + 1) * LB], gsb,
                                 tqT[hp][:, j * LB : (j + 1) * LB],
                                 start=True, stop=True)
            wsb = pgw.tile([P, LC], BF16, tag="wsb")
            nc.vector.tensor_copy(wsb, wps)
            prod = pgw.tile([P, LC], BF16, tag="prod")
            nc.vector.tensor_mul(prod, tqT[hp], wsb)
            stg_ss = pgw.tile([2, LC], F32, tag="stgss", bufs=1)
            stg_rs = pgw.tile([2, LC], F32, tag="stgrs", bufs=1)
            for j in range(2):
                r_ss = ppx.tile([2, LB], F32, tag="px")
                nc.tensor.matmul(r_ss, o2, prod[:, j * LB : (j + 1) * LB],
                                 start=True, stop=True)
                nc.vector.tensor_copy(stg_ss[:, j * LB : (j + 1) * LB], r_ss)
                r_rs = ppx.tile([2, LB], F32, tag="px")
                nc.tensor.matmul(r_rs, k2, tqT[hp][:, j * LB : (j + 1) * LB],
                                 start=True, stop=True)
                nc.vector.tensor_copy(stg_rs[:, j * LB : (j + 1) * LB], r_rs)
            nc.sync.dma_start(out=ssq_sb[2 * hp : 2 * hp + 2, :], in_=stg_ss)
            nc.sync.dma_start(out=rsum_sb[2 * hp : 2 * hp + 2, :], in_=stg_rs)

        # m = c2 * rsqrt(ssq*a - rsum^2*b + eps), vectorized over 8 heads
        nc.vector.tensor_mul(rsum_sb, rsum_sb, rsum_sb)
        nc.vector.tensor_scalar_mul(rsum_sb, rsum_sb, b_t[0:8, :])
        nc.vector.scalar_tensor_tensor(
            ssq_sb, ssq_sb, a_t[0:8, :], rsum_sb,
            op0=ALU.mult, op1=ALU.subtract)
        # m = c2 / sqrt(tau2 + eps): fast reciprocal then Sqrt
        nc.vector.tensor_scalar_add(ssq_sb, ssq_sb, EPS)
        nc.vector.reciprocal_approx_fast(rsum_sb, ssq_sb)
        nc.scalar.activation(rsum_sb, rsum_sb, AF.Sqrt,
                             bias=zero_t[0:8, :], scale=1.0)
        nc.vector.tensor_scalar_mul(rsum_sb, rsum_sb, c2[0:8, :])
        m8b = prw.tile([8, LC], BF16, tag="m8b")
        nc.vector.tensor_copy(m8b, rsum_sb)
        mdr = pdr.tile([8, LC], BF16, tag="mdr")
        nc.sync.dma_start(out=mdr[:, :], in_=m8b)

        # qts per pair: tq * m (broadcast m rows from DRAM)
        qts_l = []
        for hp in range(NHP):
            mb = pmb.tile([P, LC], BF16, tag="mb")
            for local in range(2):
                h = 2 * hp + local
                nc.sync.dma_start(out=mb[local * E : (local + 1) * E, :],
                                  in_=mdr[h : h + 1, :].to_broadcast([E, LC]))
            qts = pqts.tile([P, LC], BF16, tag="qts")
            nc.vector.tensor_mul(qts, tqT[hp], mb)
            qts_l.append(qts)

        # --- head loop: st -> exp -> V-stationary A@V -> untranspose,
        #     interleaved at kk-group granularity so PE alternates between
        #     st(h) and AV(h-1) while ACT streams exp(h) ---
        def emit_st_group(h, g):
            """st + exp for head h, kk group g; returns the at-group tile."""
            hp, local = h // 2, h % 2
            off = local * E
            tk = tkT[hp]
            at = pat.tile([P, NKG, LC], BF16, tag="at")
            for kg in range(NKG):
                kk = g * NKG + kg
                st_ps = pps.tile([P, LC], F32, tag="ps")
                for lb in range(2):
                    nc.tensor.matmul(
                        st_ps[:, lb * LB : (lb + 1) * LB],
                        tk[off : off + E, kk * P : (kk + 1) * P],
                        qts_l[hp][off : off + E, lb * LB : (lb + 1) * LB],
                        start=True, stop=True)
                nc.scalar.activation(at[:, kg, :], st_ps, AF.Exp,
                                     bias=zero_t, scale=1.0)
            return at

        def emit_av_lb(h, groups, pos, lb):
            # one dense accumulation chain per PSUM bank (no bank ping-pong)
            for kk in range(NST):
                at = groups[kk // NKG]
                nc.tensor.matmul(
                    pos[lb], va[:, kk, h, :],
                    at[:, kk % NKG, lb * LB : (lb + 1) * LB],
                    start=(kk == 0), stop=(kk == NST - 1))

        def emit_av_end(pos):
            """Drain outT, reciprocate denominator row; returns obt pair."""
            obts = []
            for lb in range(2):
                obt = pobt.tile([EV, LB], F32, tag="obt", name=f"obt{lb}")
                nc.vector.tensor_copy(obt, pos[lb])
                obts.append(obt)
            return obts

        def emit_out(h, obts):
            osb = posb.tile([P, NLT, E], F32, tag="osb")
            den = posb.tile([P, NLT], F32, tag="den")
            for half in range(2):
                obt = obts[half]
                pot = ppx.tile([P, 4, EV], F32, tag="px")
                for q4 in range(4):
                    nc.tensor.transpose(
                        pot[:, q4, :], obt[:, q4 * P : (q4 + 1) * P],
                        identf[0:EV, 0:EV])
                nc.vector.tensor_copy(
                    osb[:, half * 4 : (half + 1) * 4, :], pot[:, :, 0:E])
                nc.vector.tensor_copy(
                    den[:, half * 4 : (half + 1) * 4],
                    pot[:, :, E : E + 1].rearrange("p a c -> p (a c)"))
            nc.vector.reciprocal(den, den)
            den_b = bass.AP(
                tensor=den.tensor, offset=den.offset,
                ap=[list(den.ap[0]), list(den.ap[1]), [0, E]])
            nc.vector.tensor_mul(osb, osb, den_b)
            nc.sync.dma_start(out=o_r[:, :, h * E : (h + 1) * E], in_=osb)

        # Per head: first AV(h-1) as one dense 32-matmul PE burst (keeps the
        # HAM clock-gate warm), then the ACT-paced st+exp phase for head h.
        prev_groups = None   # at-group tiles of head h-1
        prev_obts = None     # obt pair of head h-2
        for h in range(H + 1):
            obts = None
            if h >= 1:
                pos = [ppo.tile([EV, LB], F32, tag="po", name=f"po{h}")
                       for _ in range(2)]
                for lb in range(2):
                    emit_av_lb(h - 1, prev_groups, pos, lb)
                obts = emit_av_end(pos)
            cur_groups = []
            if h < H:
                for g in range(KGRP):
                    cur_groups.append(emit_st_group(h, g))
            if h >= 2:
                emit_out(h - 2, prev_obts)
            prev_groups, prev_obts = cur_groups, obts
        emit_out(H - 1, prev_obts)

    return nc


_nc_cache = None


def kernel(queries, keys, values, attn_mask=None, directional_weights=None,
           dynamic_param=None, **_unused):
    global _nc_cache, _last_exec_time_ns
    q = np.asarray(queries, dtype=np.float32).astype(ml_dtypes.bfloat16)
    k = np.asarray(keys, dtype=np.float32).astype(ml_dtypes.bfloat16)
    v = np.asarray(values, dtype=np.float32).astype(ml_dtypes.bfloat16)
    dw = np.asarray(directional_weights, dtype=np.float32).reshape(1, 1)
    dp = np.asarray(dynamic_param, dtype=np.float32).reshape(1, 1)

    if _nc_cache is None:
        nc = build_nc()
        nc.finalize()
        _nc_cache = nc
    nc = _nc_cache

    in_maps = []
    for c in range(8):
        b, lh = c // 2, c % 2
        in_maps.append({
            "q": np.ascontiguousarray(q[b, lh * LC : (lh + 1) * LC]).reshape(LC, D),
            "k": np.ascontiguousarray(k[b]).reshape(S, D),
            "v": np.ascontiguousarray(v[b]).reshape(S, D),
            "dw": dw, "dp": dp,
        })

    tracing = bool(os.environ.get("BASS_TRACE"))
    if tracing:
        _ensure_axon_hooks()
        import concourse.bass_utils as _bu

        _orig_upload = _bu.upload_artifacts
        _bu.upload_artifacts = lambda d: d  # no bucket access in this sandbox
        try:
            res = run_bass_kernel_spmd(nc, in_maps, core_ids=list(range(8)))
        except Exception as e:  # fall back to an untraced run
            print(f"traced run failed ({e!r}); retrying untraced", file=sys.stderr)
            os.environ["BASS_NEVER_TRACE"] = "1"
            try:
                res = run_bass_kernel_spmd(nc, in_maps, core_ids=list(range(8)))
            finally:
                os.environ.pop("BASS_NEVER_TRACE", None)
        finally:
            _bu.upload_artifacts = _orig_upload
    else:
        res = run_bass_kernel_spmd(nc, in_maps, core_ids=list(range(8)))
    _last_exec_time_ns = res.exec_time_ns

    out = np.empty((B, L, H, E), dtype=np.float32)
    for c in range(8):
        b, lh = c // 2, c % 2
        out[b, lh * LC : (lh + 1) * LC] = res.results[c]["o"].reshape(LC, H, E)
    return out


# revision 35
# speedup vs baseline: 1.3605x; 1.0191x over previous
"""Dynamic Directional Attention on 8 trn2 NeuronCores (Bass/Tile), v2.

Problem: B=4, L=S=2048, H=8, E=64, f32.
  qt = tanh(q * 1/(std_H(q)+eps) * dw) * dyn     (std over the HEAD dim, ddof=1)
  kt likewise; scores[b,h,l,s] = qt . kt          (contract E)
  tau[l] = sqrt(var_s(scores[l,:], ddof=1) + eps)
  A = softmax(scale * scores / tau);  out = A @ v  [B,L,H,E]

Sharding: 8 cores = 4 batches x 2 L-halves. Each core: q[b, half] = [1024, 512],
full k/v[b] = [2048, 512]. No collectives.

v2 changes vs baseline (375us):
  - q/k/v pre-cast to bf16 on host: DMA traffic halved, no input casts on chip.
  - A@V restructured V-stationary: lhsT = Vaug [s,65], rhs = A^T streaming
    N=512. Kills the 1024x ~110ns 128-col LDWEIGHTS the A-stationary layout
    paid (~110us of pure weight-load on the PE).
  - Output produced as outT [65, l] per head (row 64 = softmax denominator),
    denominator row reciprocated in place, PE-transposed back to [l, 65],
    scaled per-partition on DVE.
  - Transform stats batched: one Rsqrt activation over all 24 chunks' vars;
    table sets: rsqrt -> exp_and_others(tanh) -> rsqrt(m) -> exp (4 loads,
    vs 8 in the baseline).
  - ksum matmuls merged into the Gram matmuls via an ones column in the
    transformed-k tiles (pair-block layout [128, 2, 4, 130]).
  - PSUM->SBUF row-stat drains via DMA instead of DVE copies.
"""

import os
import sys

for _p in ("/opt/trn_rl_repo", "/root/.axon_site/_ro/trn_rl_repo"):
    if os.path.isdir(_p) and _p not in sys.path:
        sys.path.append(_p)

import numpy as np
import ml_dtypes

import concourse.bass as bass
import concourse.mybir as mybir
import concourse.tile as tile
from concourse import bacc
from concourse.bass_utils import run_bass_kernel_spmd
from concourse.masks import make_identity

F32 = mybir.dt.float32
BF16 = mybir.dt.bfloat16
AF = mybir.ActivationFunctionType
ALU = mybir.AluOpType

B, L, S, H, E = 4, 2048, 2048, 8, 64
LC = L // 2          # 1024 l-rows per core
D = H * E            # 512 free-dim columns per core
P = 128
NLT = LC // P        # 8 l-chunks
NST = S // P         # 16 s-chunks
LB = 512
NHP = H // 2         # 4 head-pairs
EV = E + 1           # 65: V block incl ones column
BLK = 130            # tn pair-block stride: 2*64 e-cols + ones + pad
NKG = 4              # at kk-group size
KGRP = NST // NKG    # 4 groups per head
EPS = 1e-6
SCALE = 1.0 / np.sqrt(E)
UNB_H = float(H) / float(H - 1)
UNB_S = float(S) / float(S - 1)

_last_exec_time_ns = None


def _ensure_axon_hooks():
    """Provide antenv.axon_hooks (NTFF profiling hook) if the image lacks it."""
    try:
        import antenv.axon_hooks  # noqa: F401

        return
    except ImportError:
        pass
    import contextlib
    import ctypes
    import types

    try:
        import antenv
    except ImportError:
        return

    holder = {"h": None}
    mod = types.ModuleType("antenv.axon_hooks")
    mod.set_axon_ntff_profile_hook = lambda h: holder.__setitem__("h", h)
    mod.get_axon_ntff_profile_hook = lambda: holder["h"]
    sys.modules["antenv.axon_hooks"] = mod
    antenv.axon_hooks = mod

    so_path = "/opt/axon/libaxon_pjrt.so"
    if not os.path.exists(so_path):
        return
    try:
        lib = ctypes.CDLL(so_path)
    except OSError:
        return
    if not hasattr(lib, "axon_start_nrt_profile"):
        return
    lib.axon_start_nrt_profile.argtypes = [
        ctypes.POINTER(ctypes.c_int64),
        ctypes.c_size_t,
    ]
    lib.axon_start_nrt_profile.restype = ctypes.c_int64
    lib.axon_stop_nrt_profile.argtypes = [ctypes.c_char_p]
    lib.axon_stop_nrt_profile.restype = ctypes.c_int64

    @contextlib.contextmanager
    def _hook(output_dir, device_ids):
        import jax

        jax.devices()
        if device_ids:
            ids = (ctypes.c_int64 * len(device_ids))(*device_ids)
            rc = lib.axon_start_nrt_profile(ids, len(device_ids))
        else:
            rc = lib.axon_start_nrt_profile(None, 0)
        if rc != 0:
            raise RuntimeError(f"axon_start_nrt_profile rc={rc}")
        try:
            yield
        finally:
            n = lib.axon_stop_nrt_profile(str(output_dir).encode())
            print(f"profile: {n} file(s) written to {output_dir}", file=sys.stderr)

    holder["h"] = _hook


def _head_bcast(ap_2d, nh=H, ne=E):
    """View a [p, ne] AP as [p, nh, ne] with the head dim broadcast (step 0)."""
    return bass.AP(
        tensor=ap_2d.tensor,
        offset=ap_2d.offset,
        ap=[list(ap_2d.ap[0]), [0, nh], list(ap_2d.ap[1])],
    )


def build_nc():
    nc = bacc.Bacc("TRN2", target_bir_lowering=False, debug=False)
    q_d = nc.dram_tensor("q", [LC, D], BF16, kind="ExternalInput")
    k_d = nc.dram_tensor("k", [S, D], BF16, kind="ExternalInput")
    v_d = nc.dram_tensor("v", [S, D], BF16, kind="ExternalInput")
    dw_d = nc.dram_tensor("dw", [1, 1], F32, kind="ExternalInput")
    dp_d = nc.dram_tensor("dp", [1, 1], F32, kind="ExternalInput")
    sel_d = nc.dram_tensor("sel", [8, NHP * P], BF16, kind="ExternalInput")
    o_d = nc.dram_tensor("o", [LC, D], F32, kind="ExternalOutput")

    q_r = q_d.rearrange("(n p) d -> p n d", p=P)
    k_r = k_d.rearrange("(n p) d -> p n d", p=P)
    v_r = v_d.rearrange("(n p) d -> p n d", p=P)
    o_r = o_d.rearrange("(n p) d -> p n d", p=P)

    from contextlib import ExitStack

    with tile.TileContext(nc) as tc, ExitStack() as ctx:
        ek = ctx.enter_context
        sing = ek(tc.tile_pool(name="sing", bufs=1))
        pnat = ek(tc.tile_pool(name="nat", bufs=12))
        psq = ek(tc.tile_pool(name="sq", bufs=2))
        pstat = ek(tc.tile_pool(name="stat", bufs=1))
        ptn = ek(tc.tile_pool(name="tn", bufs=12))
        pqt = ek(tc.tile_pool(name="qt", bufs=4))
        pgw = ek(tc.tile_pool(name="gw", bufs=2))
        prw = ek(tc.tile_pool(name="rows", bufs=1))
        pqts = ek(tc.tile_pool(name="qts", bufs=4))
        pat = ek(tc.tile_pool(name="at", bufs=5))          # kk-group A^T bf16
        pva = ek(tc.tile_pool(name="va", bufs=1))
        pvn = ek(tc.tile_pool(name="vn", bufs=2))
        pobt = ek(tc.tile_pool(name="obt", bufs=4))        # [65,512] f32 outT
        posb = ek(tc.tile_pool(name="osb", bufs=2))
        pps = ek(tc.tile_pool(name="ps", bufs=2, space="PSUM"))    # [128,1024]
        ppo = ek(tc.tile_pool(name="po", bufs=2, space="PSUM"))    # [65,512]
        ppx = ek(tc.tile_pool(name="px", bufs=2, space="PSUM"))    # shared 2KB

        # --- constants ---
        ident = sing.tile([P, P], BF16)
        make_identity(nc, ident)
        identf = sing.tile([P, P], F32)
        make_identity(nc, identf)
        zero_t = sing.tile([P, 1], F32)
        nc.vector.memset(zero_t, 0.0)
        eps_t = sing.tile([P, 1], F32)
        nc.vector.memset(eps_t, EPS)
        dw_t = sing.tile([P, 1], F32)
        nc.sync.dma_start(out=dw_t, in_=dw_d[:, :].to_broadcast([P, 1]))
        dp_t = sing.tile([P, 1], F32)
        nc.sync.dma_start(out=dp_t, in_=dp_d[:, :].to_broadcast([P, 1]))
        dp2 = sing.tile([P, 1], F32)
        nc.vector.tensor_mul(dp2, dp_t, dp_t)
        c2 = sing.tile([P, 1], F32)  # scale * dyn^2
        nc.vector.tensor_scalar_mul(c2, dp2, float(SCALE))
        dp4 = sing.tile([P, 1], F32)
        nc.vector.tensor_mul(dp4, dp2, dp2)
        a_t = sing.tile([P, 1], F32)  # dyn^4 * UNB_S / S
        nc.vector.tensor_scalar_mul(a_t, dp4, UNB_S / S)
        b_t = sing.tile([P, 1], F32)  # dyn^4 * UNB_S / S^2
        nc.vector.tensor_scalar_mul(b_t, dp4, UNB_S / S / S)
        o2 = sing.tile([P, 2], BF16)  # per-head ones blocks for row sums
        nc.vector.memset(o2, 0.0)
        nc.vector.memset(o2[0:E, 0:1], 1.0)
        nc.vector.memset(o2[E:P, 1:2], 1.0)
        sel_t = sing.tile([8, NHP, P], BF16)
        nc.sync.dma_start(out=sel_t,
                          in_=sel_d.rearrange("r (a p) -> r a p", a=NHP))

        # --- V path: load v (bf16), pack Vaug [128, kk, h, 65] ---
        va = pva.tile([P, NST, H, EV], BF16)
        nc.vector.memset(
            va.rearrange("p a h c -> p (a h) c")[:, :, E : E + 1], 1.0)
        for kk in range(NST):
            vn = pvn.tile([P, D], BF16, tag="vn")
            nc.sync.dma_start(out=vn, in_=v_r[:, kk, :])
            nc.vector.tensor_copy(
                va[:, kk, :, 0:E],
                vn.rearrange("p (h e) -> p h e", h=H))

        # --- T1: load q/k nat pairs; per-chunk head sums into batched tiles ---
        NPAIR = (NLT + NST) // 2  # 12
        nat_pairs = []
        ssum_all = pstat.tile([P, NPAIR, 2, E], F32, tag="ssum")
        ssq_all = pstat.tile([P, NPAIR, 2, E], F32, tag="ssq")
        rstd_all = ssum_all  # reused once mean^2 is consumed by the var stt

        def t1_pair(src_r, i0, pi):
            nat = pnat.tile([P, 2, D], BF16, tag="nat")
            for u in range(2):
                nc.sync.dma_start(out=nat[:, u, :], in_=src_r[:, i0 + u, :])
            sq = psq.tile([P, 2, D], BF16, tag="sq")
            for u in range(2):
                nc.gpsimd.tensor_mul(sq[:, u, :], nat[:, u, :], nat[:, u, :])
            nc.vector.tensor_reduce(
                ssum_all[:, pi, :, :],
                nat.rearrange("p u (h e) -> p u e h", h=H),
                axis=mybir.AxisListType.X, op=ALU.add)
            nc.vector.tensor_reduce(
                ssq_all[:, pi, :, :],
                sq.rearrange("p u (h e) -> p u e h", h=H),
                axis=mybir.AxisListType.X, op=ALU.add)
            nat_pairs.append(nat)

        for j in range(NLT // 2):
            t1_pair(q_r, 2 * j, j)
        for j in range(NST // 2):
            t1_pair(k_r, 2 * j, NLT // 2 + j)

        # batched: mean^2 and biased var; rstd = rsqrt(UNB_H*var + eps)
        flat_sum = ssum_all.rearrange("p a b e -> p (a b e)")
        flat_sq = ssq_all.rearrange("p a b e -> p (a b e)")
        nc.vector.tensor_scalar_mul(flat_sum, flat_sum, 1.0 / H)
        nc.vector.tensor_mul(flat_sum, flat_sum, flat_sum)
        nc.vector.scalar_tensor_tensor(
            flat_sq, flat_sq, 1.0 / H, flat_sum,
            op0=ALU.mult, op1=ALU.subtract)
        # rstd = sqrt(1/(UNB_H*var + eps)) via fast DVE reciprocal + ACT Sqrt
        nc.vector.tensor_scalar(
            flat_sq, flat_sq, UNB_H, EPS, op0=ALU.mult, op1=ALU.add)
        flat_rstd = rstd_all.rearrange("p a b e -> p (a b e)")
        nc.vector.reciprocal_approx_fast(flat_rstd, flat_sq)
        nc.scalar.activation(flat_rstd, flat_rstd, AF.Sqrt,
                             bias=zero_t, scale=1.0)

        # --- T3: tmp = nat*rstd(bcast heads); tanh -> tn [128,2,4,130] ---
        tn_pairs = []
        for pi, nat in enumerate(nat_pairs):
            tn = ptn.tile([P, 2, NHP, BLK], BF16, tag="tn")
            for u in range(2):
                tmp = psq.tile([P, H, E], BF16, tag="tmp")
                nc.vector.tensor_mul(
                    tmp, nat[:, u, :].rearrange("p (h e) -> p h e", h=H),
                    _head_bcast(rstd_all[:, pi, u, :]))
                nc.scalar.activation(
                    tn[:, u, :, 0 : 2 * E], tmp.rearrange("p h e -> p (h e)"),
                    AF.Tanh, bias=zero_t, scale=dw_t)
            tn_pairs.append(tn)
        tn_q = tn_pairs[: NLT // 2]
        tn_k = tn_pairs[NLT // 2 :]
        for tn in tn_k:  # ones column feeding ksum via the Gram matmul
            nc.vector.memset(
                tn.rearrange("p u h c -> p (u h) c")[:, :, 2 * E : 2 * E + 1],
                1.0)

        # --- TR: PE-transpose tq/tk into [2E, l]/[2E, s] per pair ---
        tqT = [pqt.tile([P, LC], BF16, tag="tqT", name=f"tqT{i}")
               for i in range(NHP)]
        tkT = [pqt.tile([P, S], BF16, tag="tkT", name=f"tkT{i}")
               for i in range(NHP)]
        for tiles, dsts, npair in ((tn_q, tqT, NLT // 2), (tn_k, tkT, NST // 2)):
            for j in range(0, npair, 2):
                for hp in range(NHP):
                    pt = ppx.tile([P, 4, P], BF16, tag="px")
                    for w in range(2):
                        for u in range(2):
                            nc.tensor.transpose(
                                pt[:, 2 * w + u, :],
                                tiles[j + w][:, u, hp, 0 : 2 * E], ident)
                    nc.vector.tensor_copy(
                        dsts[hp][:, (2 * j) * P : (2 * j + 4) * P],
                        pt.rearrange("p a b -> p (a b)"))

        # --- G phase: block-diag Gram + ksum column; row stats. Software
        #     pipelined across pairs so the PE isn't stalled on DVE drains. ---
        ssq_sb = prw.tile([8, LC], F32, tag="ssqsb")
        rsum_sb = prw.tile([8, LC], F32, tag="rsumsb")

        def emit_g(hp):
            g_ps = ppx.tile([P, 2 * E + 1], F32, tag="px")
            for idx in range(NST):
                tn = tn_k[idx // 2]
                u = idx % 2
                nc.tensor.matmul(
                    g_ps, tn[:, u, hp, 0 : 2 * E], tn[:, u, hp, 0 : 2 * E + 1],
                    start=(idx == 0), stop=(idx == NST - 1))
            gsb = pgw.tile([P, P], BF16, tag="gsb", bufs=3)
            nc.vector.tensor_copy(gsb, g_ps[:, 0 : 2 * E])
            nc.vector.memset(gsb[0:E, E:P], 0.0)
            nc.vector.memset(gsb[E:P, 0:E], 0.0)
            k2 = pgw.tile([P, 2], BF16, tag="k2", bufs=3)
            nc.vector.memset(k2, 0.0)
            nc.vector.tensor_copy(k2[0:E, 0:1], g_ps[0:E, 2 * E : 2 * E + 1])
            nc.vector.tensor_copy(k2[E:P, 1:2], g_ps[E:P, 2 * E : 2 * E + 1])
            return gsb, k2

        def emit_w(hp, gsb):
            wps = pps.tile([P, LC], F32, tag="ps")
            for j in range(2):
                nc.tensor.matmul(wps[:, j * LB : (j + 1) * LB], gsb,
                                 tqT[hp][:, j * LB : (j + 1) * LB],
                                 start=True, stop=True)
            wsb = pgw.tile([P, LC], BF16, tag="wsb")
            nc.scalar.copy(wsb, wps)
            prod = pgw.tile([P, LC], BF16, tag="prod")
            nc.vector.tensor_mul(prod, tqT[hp], wsb)
            return prod

        def emit_rows(hp, k2, prod):
            stg_ss = pgw.tile([2, LC], F32, tag="stgss", bufs=2)
            stg_rs = pgw.tile([2, LC], F32, tag="stgrs", bufs=2)
            for j in range(2):
                r_ss = ppx.tile([2, LB], F32, tag="px")
                nc.tensor.matmul(r_ss, o2, prod[:, j * LB : (j + 1) * LB],
                                 start=True, stop=True)
                nc.vector.tensor_copy(stg_ss[:, j * LB : (j + 1) * LB], r_ss)
                r_rs = ppx.tile([2, LB], F32, tag="px")
                nc.tensor.matmul(r_rs, k2, tqT[hp][:, j * LB : (j + 1) * LB],
                                 start=True, stop=True)
                nc.vector.tensor_copy(stg_rs[:, j * LB : (j + 1) * LB], r_rs)
            nc.sync.dma_start(out=ssq_sb[2 * hp : 2 * hp + 2, :], in_=stg_ss)
            nc.sync.dma_start(out=rsum_sb[2 * hp : 2 * hp + 2, :], in_=stg_rs)

        g_state = {}
        g_state[0] = emit_g(0)
        g_state[1] = emit_g(1)
        for hp in range(NHP):
            gsb, k2 = g_state.pop(hp)
            prod = emit_w(hp, gsb)
            if hp + 2 < NHP:
                g_state[hp + 2] = emit_g(hp + 2)
            emit_rows(hp, k2, prod)

        # m = c2 * rsqrt(ssq*a - rsum^2*b + eps), vectorized over 8 heads
        nc.vector.tensor_mul(rsum_sb, rsum_sb, rsum_sb)
        nc.vector.tensor_scalar_mul(rsum_sb, rsum_sb, b_t[0:8, :])
        nc.vector.scalar_tensor_tensor(
            ssq_sb, ssq_sb, a_t[0:8, :], rsum_sb,
            op0=ALU.mult, op1=ALU.subtract)
        # m = c2 / sqrt(tau2 + eps): fast reciprocal then Sqrt
        nc.vector.tensor_scalar_add(ssq_sb, ssq_sb, EPS)
        nc.vector.reciprocal_approx_fast(rsum_sb, ssq_sb)
        nc.scalar.activation(rsum_sb, rsum_sb, AF.Sqrt,
                             bias=zero_t[0:8, :], scale=1.0)
        nc.vector.tensor_scalar_mul(rsum_sb, rsum_sb, c2[0:8, :])
        m8b = prw.tile([8, LC], BF16, tag="m8b")
        nc.vector.tensor_copy(m8b, rsum_sb)

        # qts per pair: tq * m; m broadcast to 128 partitions via a selection
        # matmul (sel[r, hp*128+p] = 1 iff r == 2hp + (p>=64))
        qts_l = []
        for hp in range(NHP):
            mb_ps = pps.tile([P, LC], F32, tag="ps")
            for j in range(2):
                nc.tensor.matmul(mb_ps[:, j * LB : (j + 1) * LB],
                                 sel_t[:, hp, :],
                                 m8b[:, j * LB : (j + 1) * LB],
                                 start=True, stop=True)
            qts = pqts.tile([P, LC], BF16, tag="qts")
            nc.vector.tensor_mul(qts, tqT[hp], mb_ps)
            qts_l.append(qts)

        # --- head loop: st -> exp -> V-stationary A@V -> untranspose,
        #     interleaved at kk-group granularity so PE alternates between
        #     st(h) and AV(h-1) while ACT streams exp(h) ---
        def emit_st_group(h, g):
            """st + exp for head h, kk group g; returns the at-group tile."""
            hp, local = h // 2, h % 2
            off = local * E
            tk = tkT[hp]
            at = pat.tile([P, NKG, LC], BF16, tag="at")
            for kg in range(NKG):
                kk = g * NKG + kg
                st_ps = pps.tile([P, LC], F32, tag="ps")
                for lb in range(2):
                    nc.tensor.matmul(
                        st_ps[:, lb * LB : (lb + 1) * LB],
                        tk[off : off + E, kk * P : (kk + 1) * P],
                        qts_l[hp][off : off + E, lb * LB : (lb + 1) * LB],
                        start=True, stop=True)
                nc.scalar.activation(at[:, kg, :], st_ps, AF.Exp,
                                     bias=zero_t, scale=1.0)
            return at

        def emit_av_lb(h, groups, pos, lb):
            # one dense accumulation chain per PSUM bank (no bank ping-pong)
            for kk in range(NST):
                at = groups[kk // NKG]
                nc.tensor.matmul(
                    pos[lb], va[:, kk, h, :],
                    at[:, kk % NKG, lb * LB : (lb + 1) * LB],
                    start=(kk == 0), stop=(kk == NST - 1))

        def emit_av_end(pos):
            """Drain outT, reciprocate denominator row; returns obt pair."""
            obts = []
            for lb in range(2):
                obt = pobt.tile([EV, LB], F32, tag="obt", name=f"obt{lb}")
                nc.vector.tensor_copy(obt, pos[lb])
                obts.append(obt)
            return obts

        def emit_out(h, obts):
            osb = posb.tile([P, NLT, E], F32, tag="osb")
            den = posb.tile([P, NLT], F32, tag="den")
            for half in range(2):
                obt = obts[half]
                pot = ppx.tile([P, 4, EV], F32, tag="px")
                for q4 in range(4):
                    nc.tensor.transpose(
                        pot[:, q4, :], obt[:, q4 * P : (q4 + 1) * P],
                        identf[0:EV, 0:EV])
                nc.vector.tensor_copy(
                    osb[:, half * 4 : (half + 1) * 4, :], pot[:, :, 0:E])
                nc.vector.tensor_copy(
                    den[:, half * 4 : (half + 1) * 4],
                    pot[:, :, E : E + 1].rearrange("p a c -> p (a c)"))
            nc.vector.reciprocal(den, den)
            den_b = bass.AP(
                tensor=den.tensor, offset=den.offset,
                ap=[list(den.ap[0]), list(den.ap[1]), [0, E]])
            nc.vector.tensor_mul(osb, osb, den_b)
            nc.sync.dma_start(out=o_r[:, :, h * E : (h + 1) * E], in_=osb)

        # Per head: first AV(h-1) as one dense 32-matmul PE burst (keeps the
        # HAM clock-gate warm), then the ACT-paced st+exp phase for head h.
        prev_groups = None   # at-group tiles of head h-1
        prev_obts = None     # obt pair of head h-2
        for h in range(H + 1):
            obts = None
            if h >= 1:
                pos = [ppo.tile([EV, LB], F32, tag="po", name=f"po{h}")
                       for _ in range(2)]
                for lb in range(2):
                    emit_av_lb(h - 1, prev_groups, pos, lb)
                obts = emit_av_end(pos)
            cur_groups = []
            if h < H:
                for g in range(KGRP):
                    cur_groups.append(emit_st_group(h, g))
            if h >= 2:
                emit_out(h - 2, prev_obts)
            prev_groups, prev_obts = cur_groups, obts
        emit_out(H - 1, prev_obts)

    return nc


_nc_cache = None


def kernel(queries, keys, values, attn_mask=None, directional_weights=None,
           dynamic_param=None, **_unused):
    global _nc_cache, _last_exec_time_ns
    q = np.asarray(queries, dtype=np.float32).astype(ml_dtypes.bfloat16)
    k = np.asarray(keys, dtype=np.float32).astype(ml_dtypes.bfloat16)
    v = np.asarray(values, dtype=np.float32).astype(ml_dtypes.bfloat16)
    dw = np.asarray(directional_weights, dtype=np.float32).reshape(1, 1)
    dp = np.asarray(dynamic_param, dtype=np.float32).reshape(1, 1)

    if _nc_cache is None:
        nc = build_nc()
        nc.finalize()
        _nc_cache = nc
    nc = _nc_cache

    sel = np.zeros((8, NHP * P), dtype=np.float32)
    for hp in range(NHP):
        sel[2 * hp, hp * P : hp * P + E] = 1.0
        sel[2 * hp + 1, hp * P + E : (hp + 1) * P] = 1.0
    sel = sel.astype(ml_dtypes.bfloat16)

    in_maps = []
    for c in range(8):
        b, lh = c // 2, c % 2
        in_maps.append({
            "q": np.ascontiguousarray(q[b, lh * LC : (lh + 1) * LC]).reshape(LC, D),
            "k": np.ascontiguousarray(k[b]).reshape(S, D),
            "v": np.ascontiguousarray(v[b]).reshape(S, D),
            "dw": dw, "dp": dp, "sel": sel,
        })

    tracing = bool(os.environ.get("BASS_TRACE"))
    if tracing:
        _ensure_axon_hooks()
        import concourse.bass_utils as _bu

        _orig_upload = _bu.upload_artifacts
        _bu.upload_artifacts = lambda d: d  # no bucket access in this sandbox
        try:
            res = run_bass_kernel_spmd(nc, in_maps, core_ids=list(range(8)))
        except Exception as e:  # fall back to an untraced run
            print(f"traced run failed ({e!r}); retrying untraced", file=sys.stderr)
            os.environ["BASS_NEVER_TRACE"] = "1"
            try:
                res = run_bass_kernel_spmd(nc, in_maps, core_ids=list(range(8)))
            finally:
                os.environ.pop("BASS_NEVER_TRACE", None)
        finally:
            _bu.upload_artifacts = _orig_upload
    else:
        res = run_bass_kernel_spmd(nc, in_maps, core_ids=list(range(8)))
    _last_exec_time_ns = res.exec_time_ns

    out = np.empty((B, L, H, E), dtype=np.float32)
    for c in range(8):
        b, lh = c // 2, c % 2
        out[b, lh * LC : (lh + 1) * LC] = res.results[c]["o"].reshape(LC, H, E)
    return out
